# revision 14
# baseline (speedup 1.0000x reference)
"""BRU (bistable recurrent unit) cell kernel for 8 Trainium2 NeuronCores.

Hardcoded problem: B=64, T=512, D=1024, U=1024, fp32.

Sharding: 8 cores = 4 batch-groups (16 batches each) x 2 unit-groups
(512 units each).  Per core the three input projections
    projT[u, token] = K[d,u].T @ xT[d, token],   token = b*512 + t
run on the PE as a 1.5-pass split:
    x @ K  =  A@K1  +  (e*2^4)@(K1*2^-4) + (A*2^-8)@(K2*2^8)
with A = fp16(x) (exact residual e = x - A in fp32) and K1 = fp16(K),
K2 = K - K1.  The main term is one fp16 matmul (1 cycle/row); the two
correction products run as ONE fp8e5 DoubleRow matmul (0.5 cycles/row,
both slot products summed in-PE), accumulating into the same fp32 PSUM
group.  Power-of-two slot scalings cancel exactly, so each slot product
lands unscaled; fp8 rounding only perturbs the (already ~2^-11) residual
terms, leaving ~1e-4 projection error at 1.5x one pass's PE cost.

The 512-step recurrence is elementwise with u on partitions, split into
two batch-group chains that are software-pipelined by emission order
(engines dispatch in-order).  Sigmoid is re-expressed via tanh so each
group needs only 2 activations per step (one merged t1/tau tanh + one
hh tanh), and 7 fused DVE ops.  Projections are chunked TC steps at a
time (double-buffered) so the PE runs ahead of the recurrence;
PSUM->SBUF copies ride on the Scalar engine, dripped between scan
steps, and fold in the bias when nonzero.
"""

import os

import numpy as np

B, T, D, U = 64, 512, 1024, 1024
NCORES = 8
NBG = 4  # batch groups
NUG = 2  # unit groups
BL = B // NBG  # 16 batches per core
UHALF = U // NUG  # 512 units per core
UH = UHALF // 128  # 4 u-chunks

_CACHE: dict = {}


def _build(T_, TC, use_memory, use_bias):
    """Build and compile the per-core Bass program."""
    import concourse.mybir as mybir
    from concourse import bacc
    from concourse.tile import TileContext

    f32 = mybir.dt.float32
    f16 = mybir.dt.float16
    f8 = mybir.dt.float8e5
    Alu = mybir.AluOpType
    Act = mybir.ActivationFunctionType
    DR = mybir.MatmulPerfMode.DoubleRow

    NTOK = BL * T_
    NCH = T_ // TC
    DC = D // 128  # 8 d-chunks

    nc = bacc.Bacc("TRN2", target_bir_lowering=False, debug=False)

    xA = nc.dram_tensor("xA", [D, NTOK], f16, kind="ExternalInput").ap()
    x8d = nc.dram_tensor("x8d", [2, D, NTOK], f8, kind="ExternalInput").ap()
    k1 = {}
    k8 = {}
    for g in "zrh":
        k1[g] = nc.dram_tensor(f"k1{g}", [D, UHALF], f16, kind="ExternalInput").ap()
        k8[g] = nc.dram_tensor(f"k8{g}", [2, D, UHALF], f8, kind="ExternalInput").ap()
    if use_memory:
        mzb = nc.dram_tensor("mzb", [128, UH, BL], f32, kind="ExternalInput").ap()
        mrb = nc.dram_tensor("mrb", [128, UH, BL], f32, kind="ExternalInput").ap()
    if use_bias:
        bts = {
            g: nc.dram_tensor(f"bt{g}", [128, UH], f32, kind="ExternalInput").ap()
            for g in "zrh"
        }
    outT = nc.dram_tensor("outT", [UHALF, NTOK], f32, kind="ExternalOutput").ap()

    xA_r = xA.rearrange("(dc p) (b t) -> dc p b t", dc=DC, b=BL)
    x8_r = x8d.rearrange("two (dc p) (b t) -> two dc p b t", dc=DC, b=BL)
    outT_r = outT.rearrange("(uh p) (b t) -> uh p b t", uh=UH, b=BL)

    # Uniform chunk schedule.  (Tapered variants — short chunks at the
    # start and/or end — were measured no better: the scan drain after the
    # last matmul is set by the scan's per-chunk rate, not chunk sizes.)
    chunks = [TC] * (T_ // TC)
    assert sum(chunks) == T_, (chunks, T_)

    with TileContext(nc) as tc:
        with (
            tc.tile_pool(name="weights", bufs=1) as wpool,
            tc.tile_pool(name="xin", bufs=2) as xpool,
            tc.tile_pool(name="proj", bufs=2) as ppool,
            tc.tile_pool(name="hout", bufs=3) as hpool,
            tc.tile_pool(name="tmp", bufs=12) as spool,
            tc.tile_pool(name="misc", bufs=1) as mpool,
            tc.tile_pool(name="psum", bufs=8, space="PSUM") as qpool,
        ):
            # Startup order: z-gate weights, then the first x chunk, then
            # the remaining weights, so the PE's first PSUM group can start
            # as early as possible.
            TC0 = chunks[0]
            w1 = {}
            w8 = {}
            for g in "zrh":
                w1[g] = wpool.tile([128, DC, UHALF], f16, tag=f"w1{g}", name=f"w1{g}")
                w8[g] = wpool.tile(
                    [128, 2, DC, UHALF], f8, tag=f"w8{g}", name=f"w8{g}"
                )
            nc.sync.dma_start(
                w1["z"][:, :, :], k1["z"].rearrange("(dc p) u -> p dc u", p=128)
            )
            xa = xpool.tile([128, DC, BL, TC], f16, tag="xa", name="xa_0")
            x8 = xpool.tile([128, 2, DC, BL, TC], f8, tag="x8", name="x8_0")
            for dc in range(DC):
                nc.sync.dma_start(xa[:, dc, :, :TC0], xA_r[dc, :, :, 0:TC0])
            for s in range(2):
                nc.sync.dma_start(
                    w8["z"][:, s, :, :],
                    k8["z"][s].rearrange("(dc p) u -> p dc u", p=128),
                )
            for dc in range(DC):
                for s in range(2):
                    nc.sync.dma_start(
                        x8[:, s, dc, :, :TC0], x8_r[s, dc, :, :, 0:TC0]
                    )
            first_x = (xa, x8)
            for g in "rh":
                nc.sync.dma_start(
                    w1[g][:, :, :], k1[g].rearrange("(dc p) u -> p dc u", p=128)
                )
                for s in range(2):
                    nc.sync.dma_start(
                        w8[g][:, s, :, :],
                        k8[g][s].rearrange("(dc p) u -> p dc u", p=128),
                    )
            if use_memory:
                # host passes mzb = 0.25*m_z, mrb = 0.5*m_r broadcasts
                mz4_t = mpool.tile([128, UH, BL], f32, tag="mz4", name="mz4")
                mr2_t = mpool.tile([128, UH, BL], f32, tag="mr2", name="mr2")
                nc.sync.dma_start(mz4_t[:, :, :], mzb[:, :, :])
                nc.sync.dma_start(mr2_t[:, :, :], mrb[:, :, :])
            if use_bias:
                b_t = {}
                for g in "zrh":
                    b_t[g] = mpool.tile([128, UH], f32, tag=f"b{g}", name=f"b{g}")
                    nc.sync.dma_start(b_t[g][:, :], bts[g][:, :])

            h0 = []
            for gi in range(2):
                h0g = mpool.tile([128, UH, BL // 2], f32, tag=f"h0{gi}", name=f"h0{gi}")
                nc.gpsimd.memset(h0g[:, :, :], 0.0)
                h0.append(h0g)

            # ---------------------------------------------------------
            # Software-pipelined scan over two batch-group chains.
            #
            # Math (per step, with carried state v = 2h):
            #   t1  = tanh(h*m_r + xr)
            #   tau = tanh(0.5*(h*m_z + xz)) so  1-z = 0.5*(1-tau)
            #   hh  = tanh(xh + (t1+1)*h)
            #   v'  = 2h' = (v/2 + hh) + tau*(v/2 - hh)
            # The host folds 0.5 into the z-gate weights/bias and halves the
            # output, so the kernel stores v.  t1 and tau come from ONE
            # merged Tanh per group (the Scalar engine dispatches serially,
            # ~270ns per instruction, so activation count dominates).
            # ---------------------------------------------------------
            GROUPS = ((0, BL // 2), (BL // 2, BL))
            HB = BL // 2

            v0t = []
            for gi in range(2):
                vg = mpool.tile([128, UH, HB], f32, tag=f"v0{gi}", name=f"v0{gi}")
                nc.gpsimd.memset(vg[:, :, :], 0.0)
                v0t.append(vg)

            def tmp(tag, gi, shape=None):
                return spool.tile(shape or [128, UH, HB], f32, tag=f"{tag}{gi}",
                                  name=f"{tag}{gi}")

            state = [dict(), dict()]

            def stage_F(gi, v, pz, pr, trel):
                """stg[0] = t1in = h*m_r + xr;  stg[1] = 0.5*zin = h*mz/2 + xz/2.
                (xz/2 is pre-folded into the z projection host-side.)"""
                s = state[gi] = {}
                s["stg"] = tmp("stg", gi, [128, 2, UH, HB])
                b0, b1 = GROUPS[gi]
                xr_t = pr[:, :, b0:b1, trel]
                xzh_t = pz[:, :, b0:b1, trel]
                if use_memory:
                    hm_r = tmp("hmr", gi)
                    hm_z = tmp("hmz", gi)
                    nc.vector.tensor_mul(hm_r[:, :, :], v, mr2_t[:, :, b0:b1])
                    nc.vector.tensor_add(s["stg"][:, 0, :, :], hm_r[:, :, :], xr_t)
                    nc.vector.tensor_mul(hm_z[:, :, :], v, mz4_t[:, :, b0:b1])
                    nc.vector.tensor_add(s["stg"][:, 1, :, :], hm_z[:, :, :], xzh_t)
                else:
                    # t1in = v*0.5 + xr ; tau_in = v*0.25 + xz/2
                    nc.vector.scalar_tensor_tensor(
                        s["stg"][:, 0, :, :], v, 0.5, xr_t, Alu.mult, Alu.add
                    )
                    nc.vector.scalar_tensor_tensor(
                        s["stg"][:, 1, :, :], v, 0.25, xzh_t, Alu.mult, Alu.add
                    )
                s["v"] = v

            def stage_X(gi):
                s = state[gi]
                s["sto"] = tmp("sto", gi, [128, 2, UH, HB])
                nc.scalar.activation(
                    s["sto"][:, :, :, :], s["stg"][:, :, :, :], Act.Tanh
                )

            def stage_M(gi, ph, trel):
                s = state[gi]
                b0, b1 = GROUPS[gi]
                xh_t = ph[:, :, b0:b1, trel]
                w = tmp("w", gi)
                # w = (t1 + 1) * v
                nc.vector.scalar_tensor_tensor(
                    w[:, :, :], s["sto"][:, 0, :, :], 1.0, s["v"], Alu.add, Alu.mult
                )
                # hhin = 0.5*w + xh = (t1+1)*h + xh
                s["hin"] = tmp("hin", gi)
                nc.vector.scalar_tensor_tensor(
                    s["hin"][:, :, :], w[:, :, :], 0.5, xh_t, Alu.mult, Alu.add
                )
                # pre-compute the hh-independent half of the blend:
                # w2 = (1+tau)*v, so v' = 0.5*w2 - (tau-1)*hh
                tau = s["sto"][:, 1, :, :]
                s["w2"] = tmp("w2", gi)
                nc.vector.scalar_tensor_tensor(
                    s["w2"][:, :, :], tau, 1.0, s["v"], Alu.add, Alu.mult
                )

            def stage_H(gi):
                s = state[gi]
                s["hh"] = tmp("hh", gi)
                nc.scalar.activation(s["hh"][:, :, :], s["hin"][:, :, :], Act.Tanh)

            def stage_B(gi, hch_g, trel):
                # v' = 0.5*w2 - (tau-1)*hh
                s = state[gi]
                hh = s["hh"][:, :, :]
                r1 = tmp("r1", gi)
                nc.vector.scalar_tensor_tensor(
                    r1[:, :, :], s["sto"][:, 1, :, :], 1.0, hh,
                    Alu.subtract, Alu.mult,
                )
                nc.vector.scalar_tensor_tensor(
                    hch_g[:, :, :, trel], s["w2"][:, :, :], 0.5, r1[:, :, :],
                    Alu.mult, Alu.subtract,
                )

            def emit_matmuls(c, TCc, xa, x8):
                projs = {}
                copies = []
                for g in "zrh":
                    pg = ppool.tile(
                        [128, UH, BL, TC], f32, tag=f"p{g}", name=f"p{g}_{c}"
                    )
                    projs[g] = pg
                    for uh in range(UH):
                        us = slice(uh * 128, (uh + 1) * 128)
                        ps = qpool.tile([128, BL, TC], f32, tag="ps")
                        for dc in range(DC):
                            nc.tensor.matmul(
                                ps[:, :, :TCc], w1[g][:, dc, us], xa[:, dc, :, :TCc],
                                start=(dc == 0), stop=False,
                            )
                        for dc in range(DC):
                            nc.tensor.matmul(
                                ps[:, :, :TCc], w8[g][:, :, dc, us],
                                x8[:, :, dc, :, :TCc],
                                start=False, stop=(dc == DC - 1), perf_mode=DR,
                            )

                        def mkcopy(pg=pg, uh=uh, ps=ps, g=g):
                            def do():
                                if use_bias:
                                    nc.scalar.activation(
                                        pg[:, uh, :, :TCc], ps[:, :, :TCc],
                                        Act.Identity, bias=b_t[g][:, uh : uh + 1],
                                    )
                                else:
                                    nc.scalar.activation(
                                        pg[:, uh, :, :TCc], ps[:, :, :TCc],
                                        Act.Identity,
                                    )
                            return do

                        copies.append(mkcopy())
                return projs, copies

            def emit_scan(sc, TCsc, projs, prev_v, prev_tc, pending):
                pz, pr, ph = projs["z"], projs["r"], projs["h"]
                hch = [
                    hpool.tile([128, UH, HB, TC], f32, tag=f"hch{gi}",
                               name=f"hch{gi}_{sc}")
                    for gi in range(2)
                ]

                def v_of(gi, trel):
                    if trel == 0:
                        if sc == 0:
                            return v0t[gi][:, :, :]
                        return prev_v[gi][:, :, :, prev_tc - 1]
                    return hch[gi][:, :, :, trel - 1]

                ncopies = len(pending)
                emitted = 0

                def drip(trel):
                    nonlocal emitted
                    want = ((trel + 1) * ncopies) // max(TCsc - 1, 1)
                    while emitted < min(want, ncopies):
                        pending[emitted]()
                        emitted += 1

                def mids(trel):
                    stage_M(0, ph, trel)
                    stage_X(1)
                    stage_M(1, ph, trel)
                    stage_H(0)
                    stage_H(1)

                # prologue (trel = 0)
                stage_F(0, v_of(0, 0), pz, pr, 0)
                stage_X(0)
                stage_F(1, v_of(1, 0), pz, pr, 0)
                mids(0)
                for trel in range(1, TCsc):
                    cur0, cur1 = state[0], state[1]
                    stage_B(0, hch[0], trel - 1)
                    stage_F(0, v_of(0, trel), pz, pr, trel)
                    new0 = state[0]
                    stage_X(0)
                    state[0], state[1] = cur0, cur1
                    stage_B(1, hch[1], trel - 1)
                    state[0] = new0
                    stage_F(1, v_of(1, trel), pz, pr, trel)
                    mids(trel)
                    drip(trel - 1)
                stage_B(0, hch[0], TCsc - 1)
                stage_B(1, hch[1], TCsc - 1)
                while emitted < ncopies:
                    pending[emitted]()
                    emitted += 1
                return hch

            # main pipeline over chunks
            prev_v = None
            prev_tc = None
            prev_projs = None
            t0 = 0
            t0s = []
            for c, TCc in enumerate(chunks):
                if c == 0:
                    xa, x8 = first_x
                else:
                    xa = xpool.tile([128, DC, BL, TC], f16, tag="xa", name=f"xa_{c}")
                    x8 = xpool.tile(
                        [128, 2, DC, BL, TC], f8, tag="x8", name=f"x8_{c}"
                    )
                    for dc in range(DC):
                        nc.sync.dma_start(
                            xa[:, dc, :, :TCc], xA_r[dc, :, :, t0 : t0 + TCc]
                        )
                        for s in range(2):
                            nc.sync.dma_start(
                                x8[:, s, dc, :, :TCc],
                                x8_r[s, dc, :, :, t0 : t0 + TCc],
                            )
                projs, copies = emit_matmuls(c, TCc, xa, x8)
                if c == 0:
                    for do in copies:
                        do()
                else:
                    sc = c - 1
                    TCsc = chunks[sc]
                    hch = emit_scan(sc, TCsc, prev_projs, prev_v, prev_tc, copies)
                    for uh in range(UH):
                        for gi, (b0, b1) in enumerate(GROUPS):
                            nc.sync.dma_start(
                                outT_r[uh, :, b0:b1, t0s[sc] : t0s[sc] + TCsc],
                                hch[gi][:, uh, :, :TCsc],
                            )
                    prev_v = hch
                    prev_tc = TCsc
                prev_projs = projs
                t0s.append(t0)
                t0 += TCc
            sc = len(chunks) - 1
            TCsc = chunks[sc]
            hch = emit_scan(sc, TCsc, prev_projs, prev_v, prev_tc, [])
            for uh in range(UH):
                for gi, (b0, b1) in enumerate(GROUPS):
                    nc.sync.dma_start(
                        outT_r[uh, :, b0:b1, t0s[sc] : t0s[sc] + TCsc],
                        hch[gi][:, uh, :, :TCsc],
                    )

    nc.compile()
    return nc


def _get_nc(T_, TC, use_memory, use_bias):
    key = (T_, TC, use_memory, use_bias)
    if key not in _CACHE:
        _CACHE[key] = _build(T_, TC, use_memory, use_bias)
    return _CACHE[key]


def kernel(
    x,
    kernel_z,
    kernel_r,
    kernel_h,
    memory_z,
    memory_r,
    bias_z,
    bias_r,
    bias_h,
):
    from concourse import bass_utils

    x = np.asarray(x, dtype=np.float32)
    Ks = {
        "z": np.asarray(kernel_z, dtype=np.float32),
        "r": np.asarray(kernel_r, dtype=np.float32),
        "h": np.asarray(kernel_h, dtype=np.float32),
    }
    mem = {
        "z": np.asarray(memory_z, dtype=np.float32),
        "r": np.asarray(memory_r, dtype=np.float32),
    }
    bias = {
        "z": np.asarray(bias_z, dtype=np.float32),
        "r": np.asarray(bias_r, dtype=np.float32),
        "h": np.asarray(bias_h, dtype=np.float32),
    }

    B_, T_, D_ = x.shape
    assert (B_, D_) == (B, D), (x.shape,)
    TC = int(os.environ.get("BRU_TC", "32"))

    use_memory = not all(np.all(m == 1.0) for m in mem.values())
    use_bias = not all(np.all(b == 0.0) for b in bias.values())

    nc = _get_nc(T_, TC, use_memory, use_bias)

    import ml_dtypes

    f8e5 = ml_dtypes.float8_e5m2

    # Split weights once (shared across cores).  The z-gate weights/bias are
    # pre-halved: the kernel computes tau = tanh(0.5*zin) instead of
    # sigmoid(zin).  Each gate ships the fp16 main K1 plus a DoubleRow fp8
    # pair [K1*2^-4, K2*2^8] whose slot scalings cancel against the fp8
    # moving pair [e*2^4, A*2^-8].
    w1_full = {}
    w8_full = {}
    for g, K in Ks.items():
        if g == "z":
            K = K * np.float32(0.5)
        K1 = K.astype(np.float16)
        K2 = K - K1.astype(np.float32)
        k8 = np.empty((2, D, K.shape[1]), dtype=f8e5)
        k8[0] = (K1.astype(np.float32) * np.float32(2.0 ** -4)).astype(f8e5)
        k8[1] = (K2 * np.float32(2.0 ** 8)).astype(f8e5)
        w1_full[g] = K1
        w8_full[g] = k8

    in_maps = []
    for c in range(NCORES):
        bg, ug = divmod(c, NUG)
        xc = x[bg * BL : (bg + 1) * BL].reshape(BL * T_, D)
        xcT = np.ascontiguousarray(xc.T)  # [D, NTOK] fp32
        A = xcT.astype(np.float16)
        e = xcT - A.astype(np.float32)
        x8 = np.empty((2, D, xcT.shape[1]), dtype=f8e5)
        x8[0] = (e * np.float32(16.0)).astype(f8e5)
        x8[1] = (A.astype(np.float32) * np.float32(2.0 ** -8)).astype(f8e5)
        us = slice(ug * UHALF, (ug + 1) * UHALF)
        m = {"xA": A, "x8d": x8}
        for g in "zrh":
            m[f"k1{g}"] = np.ascontiguousarray(w1_full[g][:, us])
            m[f"k8{g}"] = np.ascontiguousarray(w8_full[g][:, :, us])
        if use_memory:
            # element (p, uh, b) = mem[ug*UHALF + uh*128 + p], pre-scaled
            for name, v, sc_ in (
                ("mzb", mem["z"], 0.25),
                ("mrb", mem["r"], 0.5),
            ):
                mv = (v[us] * np.float32(sc_)).reshape(UH, 128).T  # [128, UH]
                m[name] = np.ascontiguousarray(
                    np.broadcast_to(mv[:, :, None], (128, UH, BL))
                )
        if use_bias:
            for g in "zrh":
                bv = bias[g][us]
                if g == "z":
                    bv = bv * np.float32(0.5)
                m[f"bt{g}"] = np.ascontiguousarray(bv.reshape(UH, 128).T)
        in_maps.append(m)

    res = bass_utils.run_bass_kernel_spmd(nc, in_maps, core_ids=list(range(NCORES)))

    out = np.empty((B, T_, U), dtype=np.float32)
    for c in range(NCORES):
        bg, ug = divmod(c, NUG)
        oT = res.results[c]["outT"]  # [UHALF, BL*T_] holding v = 2h
        out[bg * BL : (bg + 1) * BL, :, ug * UHALF : (ug + 1) * UHALF] = (
            oT.reshape(UHALF, BL, T_).transpose(1, 2, 0)
        )
    out *= np.float32(0.5)
    return out



# revision 19
# speedup vs baseline: 1.0076x; 1.0076x over previous
"""BRU (bistable recurrent unit) cell kernel for 8 Trainium2 NeuronCores.

Hardcoded problem: B=64, T=512, D=1024, U=1024, fp32.

Sharding: 8 cores = 4 batch-groups (16 batches each) x 2 unit-groups
(512 units each).  Per core the three input projections
    projT[u, token] = K[d,u].T @ xT[d, token],   token = b*512 + t
run on the PE as a 1.5-pass split:
    x @ K  =  A@K1  +  (e*2^4)@(K1*2^-4) + (A*2^-8)@(K2*2^8)
with A = fp16(x) (exact residual e = x - A in fp32) and K1 = fp16(K),
K2 = K - K1.  The main term is one fp16 matmul (1 cycle/row); the two
correction products run as ONE fp8e5 DoubleRow matmul (0.5 cycles/row,
both slot products summed in-PE), accumulating into the same fp32 PSUM
group.  Power-of-two slot scalings cancel exactly, so each slot product
lands unscaled; fp8 rounding only perturbs the (already ~2^-11) residual
terms, leaving ~1e-4 projection error at 1.5x one pass's PE cost.

The 512-step recurrence is elementwise with u on partitions, split into
two batch-group chains that are software-pipelined by emission order
(engines dispatch in-order).  Sigmoid is re-expressed via tanh so each
group needs only 2 activations per step (one merged t1/tau tanh + one
hh tanh), and 7 fused DVE ops.  Projections are chunked TC steps at a
time (double-buffered) so the PE runs ahead of the recurrence;
PSUM->SBUF copies ride on the Scalar engine, dripped between scan
steps, and fold in the bias when nonzero.
"""

import os

import numpy as np

B, T, D, U = 64, 512, 1024, 1024
NCORES = 8
NBG = 4  # batch groups
NUG = 2  # unit groups
BL = B // NBG  # 16 batches per core
UHALF = U // NUG  # 512 units per core
UH = UHALF // 128  # 4 u-chunks

_CACHE: dict = {}


def _build(T_, TC, use_memory, use_bias):
    """Build and compile the per-core Bass program."""
    import concourse.mybir as mybir
    from concourse import bacc
    from concourse.tile import TileContext

    f32 = mybir.dt.float32
    f16 = mybir.dt.float16
    f8 = mybir.dt.float8e5
    Alu = mybir.AluOpType
    Act = mybir.ActivationFunctionType
    DR = mybir.MatmulPerfMode.DoubleRow

    NTOK = BL * T_
    NCH = T_ // TC
    DC = D // 128  # 8 d-chunks

    nc = bacc.Bacc("TRN2", target_bir_lowering=False, debug=False)

    xA = nc.dram_tensor("xA", [D, NTOK], f16, kind="ExternalInput").ap()
    x8d = nc.dram_tensor("x8d", [2, D, NTOK], f8, kind="ExternalInput").ap()
    k1 = {}
    k8 = {}
    for g in "zrh":
        k1[g] = nc.dram_tensor(f"k1{g}", [D, UHALF], f16, kind="ExternalInput").ap()
        k8[g] = nc.dram_tensor(f"k8{g}", [2, D, UHALF], f8, kind="ExternalInput").ap()
    if use_memory:
        mzb = nc.dram_tensor("mzb", [128, UH, BL], f32, kind="ExternalInput").ap()
        mrb = nc.dram_tensor("mrb", [128, UH, BL], f32, kind="ExternalInput").ap()
    if use_bias:
        bts = {
            g: nc.dram_tensor(f"bt{g}", [128, UH], f32, kind="ExternalInput").ap()
            for g in "zrh"
        }
    outT = nc.dram_tensor("outT", [UHALF, NTOK], f32, kind="ExternalOutput").ap()

    xA_r = xA.rearrange("(dc p) (b t) -> dc p b t", dc=DC, b=BL)
    x8_r = x8d.rearrange("two (dc p) (b t) -> two dc p b t", dc=DC, b=BL)
    outT_r = outT.rearrange("(uh p) (b t) -> uh p b t", uh=UH, b=BL)

    # Uniform chunk schedule.  (Tapered variants — short chunks at the
    # start and/or end — were measured no better: the scan drain after the
    # last matmul is set by the scan's per-chunk rate, not chunk sizes.)
    chunks = [TC] * (T_ // TC)
    assert sum(chunks) == T_, (chunks, T_)

    with TileContext(nc) as tc:
        with (
            tc.tile_pool(name="weights", bufs=1) as wpool,
            tc.tile_pool(name="xin", bufs=2) as xpool,
            tc.tile_pool(name="proj", bufs=2) as ppool,
            tc.tile_pool(name="hout", bufs=3) as hpool,
            tc.tile_pool(name="tmp", bufs=12) as spool,
            tc.tile_pool(name="misc", bufs=1) as mpool,
            tc.tile_pool(name="psum", bufs=8, space="PSUM") as qpool,
        ):
            # Startup order: z-gate weights, then the first x chunk, then
            # the remaining weights, so the PE's first PSUM group can start
            # as early as possible.
            TC0 = chunks[0]
            w1 = {}
            w8 = {}
            for g in "zrh":
                w1[g] = wpool.tile([128, DC, UHALF], f16, tag=f"w1{g}", name=f"w1{g}")
                w8[g] = wpool.tile(
                    [128, 2, DC, UHALF], f8, tag=f"w8{g}", name=f"w8{g}"
                )
            nc.sync.dma_start(
                w1["z"][:, :, :], k1["z"].rearrange("(dc p) u -> p dc u", p=128)
            )
            xa = xpool.tile([128, DC, BL, TC], f16, tag="xa", name="xa_0")
            x8 = xpool.tile([128, 2, DC, BL, TC], f8, tag="x8", name="x8_0")
            for dc in range(DC):
                nc.sync.dma_start(xa[:, dc, :, :TC0], xA_r[dc, :, :, 0:TC0])
            for s in range(2):
                nc.sync.dma_start(
                    w8["z"][:, s, :, :],
                    k8["z"][s].rearrange("(dc p) u -> p dc u", p=128),
                )
            for dc in range(DC):
                for s in range(2):
                    nc.sync.dma_start(
                        x8[:, s, dc, :, :TC0], x8_r[s, dc, :, :, 0:TC0]
                    )
            first_x = (xa, x8)
            for g in "rh":
                nc.sync.dma_start(
                    w1[g][:, :, :], k1[g].rearrange("(dc p) u -> p dc u", p=128)
                )
                for s in range(2):
                    nc.sync.dma_start(
                        w8[g][:, s, :, :],
                        k8[g][s].rearrange("(dc p) u -> p dc u", p=128),
                    )
            if use_memory:
                # host passes mzb = 0.25*m_z, mrb = 0.5*m_r broadcasts
                mz4_t = mpool.tile([128, UH, BL], f32, tag="mz4", name="mz4")
                mr2_t = mpool.tile([128, UH, BL], f32, tag="mr2", name="mr2")
                nc.sync.dma_start(mz4_t[:, :, :], mzb[:, :, :])
                nc.sync.dma_start(mr2_t[:, :, :], mrb[:, :, :])
            if use_bias:
                b_t = {}
                for g in "zrh":
                    b_t[g] = mpool.tile([128, UH], f32, tag=f"b{g}", name=f"b{g}")
                    nc.sync.dma_start(b_t[g][:, :], bts[g][:, :])

            h0 = []
            for gi in range(2):
                h0g = mpool.tile([128, UH, BL // 2], f32, tag=f"h0{gi}", name=f"h0{gi}")
                nc.gpsimd.memset(h0g[:, :, :], 0.0)
                h0.append(h0g)

            # ---------------------------------------------------------
            # Software-pipelined scan over two batch-group chains.
            #
            # Math (per step, with carried state v = 2h):
            #   t1  = tanh(h*m_r + xr)
            #   tau = tanh(0.5*(h*m_z + xz)) so  1-z = 0.5*(1-tau)
            #   hh  = tanh(xh + (t1+1)*h)
            #   v'  = 2h' = (v/2 + hh) + tau*(v/2 - hh)
            # The host folds 0.5 into the z-gate weights/bias and halves the
            # output, so the kernel stores v.
            #
            # The wall-clock of the scan is 512 x the per-chain serial step
            # latency (chains are batch splits; each runs all T steps), so
            # the emission is latency-driven: t1 and tau get SEPARATE Tanh
            # instructions so that only t1 sits on the serial path
            # v' -> t1in -> t1 -> w -> hin -> hh -> r1 -> v'; tau's Tanh and
            # the w2 blend ride in the act/DVE slack between path ops.  The
            # Activation engine's SBUF-ack (~185ns) is charged to every
            # cross-engine consumer, so each removed act visit saves ~450ns
            # of path.
            # ---------------------------------------------------------
            GROUPS = ((0, BL // 2), (BL // 2, BL))
            HB = BL // 2

            v0t = []
            for gi in range(2):
                vg = mpool.tile([128, UH, HB], f32, tag=f"v0{gi}", name=f"v0{gi}")
                nc.gpsimd.memset(vg[:, :, :], 0.0)
                v0t.append(vg)

            def tmp(tag, gi, shape=None):
                return spool.tile(shape or [128, UH, HB], f32, tag=f"{tag}{gi}",
                                  name=f"{tag}{gi}")

            state = [dict(), dict()]

            def stage_F(gi, v, pz, pr, trel):
                """ta = t1in = h*m_r + xr;  tb = 0.5*zin = h*mz/2 + xz/2.
                (xz/2 is pre-folded into the z projection host-side.)"""
                s = state[gi] = {}
                s["ta"] = tmp("ta", gi)
                s["tb"] = tmp("tb", gi)
                b0, b1 = GROUPS[gi]
                xr_t = pr[:, :, b0:b1, trel]
                xzh_t = pz[:, :, b0:b1, trel]
                if use_memory:
                    hm_r = tmp("hmr", gi)
                    hm_z = tmp("hmz", gi)
                    nc.vector.tensor_mul(hm_r[:, :, :], v, mr2_t[:, :, b0:b1])
                    nc.vector.tensor_add(s["ta"][:, :, :], hm_r[:, :, :], xr_t)
                    nc.vector.tensor_mul(hm_z[:, :, :], v, mz4_t[:, :, b0:b1])
                    nc.vector.tensor_add(s["tb"][:, :, :], hm_z[:, :, :], xzh_t)
                else:
                    # t1in = v*0.5 + xr ; tau_in = v*0.25 + xz/2
                    nc.vector.scalar_tensor_tensor(
                        s["ta"][:, :, :], v, 0.5, xr_t, Alu.mult, Alu.add
                    )
                    nc.vector.scalar_tensor_tensor(
                        s["tb"][:, :, :], v, 0.25, xzh_t, Alu.mult, Alu.add
                    )
                s["v"] = v

            def stage_Xa(gi):
                s = state[gi]
                s["t1"] = tmp("t1", gi)
                nc.scalar.activation(s["t1"][:, :, :], s["ta"][:, :, :], Act.Tanh)

            def stage_Xb(gi):
                s = state[gi]
                s["tau"] = tmp("tau", gi)
                nc.scalar.activation(s["tau"][:, :, :], s["tb"][:, :, :], Act.Tanh)

            def stage_M(gi, ph, trel):
                s = state[gi]
                b0, b1 = GROUPS[gi]
                xh_t = ph[:, :, b0:b1, trel]
                w = tmp("w", gi)
                # w = (t1 + 1) * v
                nc.vector.scalar_tensor_tensor(
                    w[:, :, :], s["t1"][:, :, :], 1.0, s["v"], Alu.add, Alu.mult
                )
                # hhin = 0.5*w + xh = (t1+1)*h + xh
                s["hin"] = tmp("hin", gi)
                nc.vector.scalar_tensor_tensor(
                    s["hin"][:, :, :], w[:, :, :], 0.5, xh_t, Alu.mult, Alu.add
                )

            def stage_W2(gi):
                # hh-independent half of the blend: w2 = (1+tau)*v, so
                # v' = 0.5*w2 - (tau-1)*hh.  Off the serial path.
                s = state[gi]
                s["w2"] = tmp("w2", gi)
                nc.vector.scalar_tensor_tensor(
                    s["w2"][:, :, :], s["tau"][:, :, :], 1.0, s["v"],
                    Alu.add, Alu.mult,
                )

            def stage_H(gi):
                s = state[gi]
                s["hh"] = tmp("hh", gi)
                nc.scalar.activation(s["hh"][:, :, :], s["hin"][:, :, :], Act.Tanh)

            def stage_B(gi, hch_g, trel):
                # v' = 0.5*w2 - (tau-1)*hh
                s = state[gi]
                hh = s["hh"][:, :, :]
                r1 = tmp("r1", gi)
                nc.vector.scalar_tensor_tensor(
                    r1[:, :, :], s["tau"][:, :, :], 1.0, hh,
                    Alu.subtract, Alu.mult,
                )
                nc.vector.scalar_tensor_tensor(
                    hch_g[:, :, :, trel], s["w2"][:, :, :], 0.5, r1[:, :, :],
                    Alu.mult, Alu.subtract,
                )

            def emit_matmuls(c, TCc, xa, x8):
                projs = {}
                copies = []
                for g in "zrh":
                    pg = ppool.tile(
                        [128, UH, BL, TC], f32, tag=f"p{g}", name=f"p{g}_{c}"
                    )
                    projs[g] = pg
                    for uh in range(UH):
                        us = slice(uh * 128, (uh + 1) * 128)
                        ps = qpool.tile([128, BL, TC], f32, tag="ps")
                        for dc in range(DC):
                            nc.tensor.matmul(
                                ps[:, :, :TCc], w1[g][:, dc, us], xa[:, dc, :, :TCc],
                                start=(dc == 0), stop=False,
                            )
                        for dc in range(DC):
                            nc.tensor.matmul(
                                ps[:, :, :TCc], w8[g][:, :, dc, us],
                                x8[:, :, dc, :, :TCc],
                                start=False, stop=(dc == DC - 1), perf_mode=DR,
                            )

                        def mkcopy(pg=pg, uh=uh, ps=ps, g=g, ci=len(copies)):
                            def do():
                                if use_bias:
                                    # bias fold needs the act engine
                                    nc.scalar.activation(
                                        pg[:, uh, :, :TCc], ps[:, :, :TCc],
                                        Act.Identity, bias=b_t[g][:, uh : uh + 1],
                                    )
                                elif ci % 2 == 0:
                                    # Pool can't read PSUM and DMA can't
                                    # source it, so alternate the drains
                                    # between the two queues that can —
                                    # halves the head-of-line pollution each
                                    # queue inflicts on the scan's path ops.
                                    nc.scalar.activation(
                                        pg[:, uh, :, :TCc], ps[:, :, :TCc],
                                        Act.Identity,
                                    )
                                else:
                                    nc.vector.tensor_copy(
                                        pg[:, uh, :, :TCc], ps[:, :, :TCc]
                                    )
                            return do

                        copies.append(mkcopy())
                return projs, copies

            def emit_scan(sc, TCsc, projs, prev_v, prev_tc, pending):
                pz, pr, ph = projs["z"], projs["r"], projs["h"]
                hch = [
                    hpool.tile([128, UH, HB, TC], f32, tag=f"hch{gi}",
                               name=f"hch{gi}_{sc}")
                    for gi in range(2)
                ]

                def v_of(gi, trel):
                    if trel == 0:
                        if sc == 0:
                            return v0t[gi][:, :, :]
                        return prev_v[gi][:, :, :, prev_tc - 1]
                    return hch[gi][:, :, :, trel - 1]

                ncopies = len(pending)
                emitted = 0

                def drip(trel):
                    nonlocal emitted
                    want = ((trel + 1) * ncopies) // max(TCsc - 1, 1)
                    while emitted < min(want, ncopies):
                        pending[emitted]()
                        emitted += 1

                def mids(trel):
                    # chain0 path segment [w,hin] -> hh while chain1's t1/tau
                    # acts and w2 blends fill the act/DVE slack
                    stage_M(0, ph, trel)
                    stage_Xa(1)
                    stage_H(0)
                    stage_W2(0)
                    stage_M(1, ph, trel)
                    stage_Xb(1)
                    stage_H(1)
                    stage_W2(1)

                # prologue (trel = 0)
                stage_F(0, v_of(0, 0), pz, pr, 0)
                stage_Xa(0)
                stage_Xb(0)
                stage_F(1, v_of(1, 0), pz, pr, 0)
                mids(0)
                for trel in range(1, TCsc):
                    cur0, cur1 = state[0], state[1]
                    stage_B(0, hch[0], trel - 1)
                    stage_F(0, v_of(0, trel), pz, pr, trel)
                    new0 = state[0]
                    stage_Xa(0)
                    stage_Xb(0)
                    state[0], state[1] = cur0, cur1
                    stage_B(1, hch[1], trel - 1)
                    state[0] = new0
                    stage_F(1, v_of(1, trel), pz, pr, trel)
                    mids(trel)
                    drip(trel - 1)
                stage_B(0, hch[0], TCsc - 1)
                stage_B(1, hch[1], TCsc - 1)
                while emitted < ncopies:
                    pending[emitted]()
                    emitted += 1
                return hch

            # main pipeline over chunks
            prev_v = None
            prev_tc = None
            prev_projs = None
            t0 = 0
            t0s = []
            for c, TCc in enumerate(chunks):
                if c == 0:
                    xa, x8 = first_x
                else:
                    xa = xpool.tile([128, DC, BL, TC], f16, tag="xa", name=f"xa_{c}")
                    x8 = xpool.tile(
                        [128, 2, DC, BL, TC], f8, tag="x8", name=f"x8_{c}"
                    )
                    for dc in range(DC):
                        nc.sync.dma_start(
                            xa[:, dc, :, :TCc], xA_r[dc, :, :, t0 : t0 + TCc]
                        )
                        for s in range(2):
                            nc.sync.dma_start(
                                x8[:, s, dc, :, :TCc],
                                x8_r[s, dc, :, :, t0 : t0 + TCc],
                            )
                projs, copies = emit_matmuls(c, TCc, xa, x8)
                if c == 0:
                    for do in copies:
                        do()
                else:
                    sc = c - 1
                    TCsc = chunks[sc]
                    hch = emit_scan(sc, TCsc, prev_projs, prev_v, prev_tc, copies)
                    for uh in range(UH):
                        for gi, (b0, b1) in enumerate(GROUPS):
                            nc.sync.dma_start(
                                outT_r[uh, :, b0:b1, t0s[sc] : t0s[sc] + TCsc],
                                hch[gi][:, uh, :, :TCsc],
                            )
                    prev_v = hch
                    prev_tc = TCsc
                prev_projs = projs
                t0s.append(t0)
                t0 += TCc
            sc = len(chunks) - 1
            TCsc = chunks[sc]
            hch = emit_scan(sc, TCsc, prev_projs, prev_v, prev_tc, [])
            for uh in range(UH):
                for gi, (b0, b1) in enumerate(GROUPS):
                    nc.sync.dma_start(
                        outT_r[uh, :, b0:b1, t0s[sc] : t0s[sc] + TCsc],
                        hch[gi][:, uh, :, :TCsc],
                    )

    nc.compile()
    return nc


def _get_nc(T_, TC, use_memory, use_bias):
    key = (T_, TC, use_memory, use_bias)
    if key not in _CACHE:
        _CACHE[key] = _build(T_, TC, use_memory, use_bias)
    return _CACHE[key]


def kernel(
    x,
    kernel_z,
    kernel_r,
    kernel_h,
    memory_z,
    memory_r,
    bias_z,
    bias_r,
    bias_h,
):
    from concourse import bass_utils

    x = np.asarray(x, dtype=np.float32)
    Ks = {
        "z": np.asarray(kernel_z, dtype=np.float32),
        "r": np.asarray(kernel_r, dtype=np.float32),
        "h": np.asarray(kernel_h, dtype=np.float32),
    }
    mem = {
        "z": np.asarray(memory_z, dtype=np.float32),
        "r": np.asarray(memory_r, dtype=np.float32),
    }
    bias = {
        "z": np.asarray(bias_z, dtype=np.float32),
        "r": np.asarray(bias_r, dtype=np.float32),
        "h": np.asarray(bias_h, dtype=np.float32),
    }

    B_, T_, D_ = x.shape
    assert (B_, D_) == (B, D), (x.shape,)
    TC = int(os.environ.get("BRU_TC", "32"))

    use_memory = not all(np.all(m == 1.0) for m in mem.values())
    use_bias = not all(np.all(b == 0.0) for b in bias.values())

    nc = _get_nc(T_, TC, use_memory, use_bias)

    import ml_dtypes

    f8e5 = ml_dtypes.float8_e5m2

    # Split weights once (shared across cores).  The z-gate weights/bias are
    # pre-halved: the kernel computes tau = tanh(0.5*zin) instead of
    # sigmoid(zin).  Each gate ships the fp16 main K1 plus a DoubleRow fp8
    # pair [K1*2^-4, K2*2^8] whose slot scalings cancel against the fp8
    # moving pair [e*2^4, A*2^-8].
    w1_full = {}
    w8_full = {}
    for g, K in Ks.items():
        if g == "z":
            K = K * np.float32(0.5)
        K1 = K.astype(np.float16)
        K2 = K - K1.astype(np.float32)
        k8 = np.empty((2, D, K.shape[1]), dtype=f8e5)
        k8[0] = (K1.astype(np.float32) * np.float32(2.0 ** -4)).astype(f8e5)
        k8[1] = (K2 * np.float32(2.0 ** 8)).astype(f8e5)
        w1_full[g] = K1
        w8_full[g] = k8

    in_maps = []
    for c in range(NCORES):
        bg, ug = divmod(c, NUG)
        xc = x[bg * BL : (bg + 1) * BL].reshape(BL * T_, D)
        xcT = np.ascontiguousarray(xc.T)  # [D, NTOK] fp32
        A = xcT.astype(np.float16)
        e = xcT - A.astype(np.float32)
        x8 = np.empty((2, D, xcT.shape[1]), dtype=f8e5)
        x8[0] = (e * np.float32(16.0)).astype(f8e5)
        x8[1] = (A.astype(np.float32) * np.float32(2.0 ** -8)).astype(f8e5)
        us = slice(ug * UHALF, (ug + 1) * UHALF)
        m = {"xA": A, "x8d": x8}
        for g in "zrh":
            m[f"k1{g}"] = np.ascontiguousarray(w1_full[g][:, us])
            m[f"k8{g}"] = np.ascontiguousarray(w8_full[g][:, :, us])
        if use_memory:
            # element (p, uh, b) = mem[ug*UHALF + uh*128 + p], pre-scaled
            for name, v, sc_ in (
                ("mzb", mem["z"], 0.25),
                ("mrb", mem["r"], 0.5),
            ):
                mv = (v[us] * np.float32(sc_)).reshape(UH, 128).T  # [128, UH]
                m[name] = np.ascontiguousarray(
                    np.broadcast_to(mv[:, :, None], (128, UH, BL))
                )
        if use_bias:
            for g in "zrh":
                bv = bias[g][us]
                if g == "z":
                    bv = bv * np.float32(0.5)
                m[f"bt{g}"] = np.ascontiguousarray(bv.reshape(UH, 128).T)
        in_maps.append(m)

    res = bass_utils.run_bass_kernel_spmd(nc, in_maps, core_ids=list(range(NCORES)))

    out = np.empty((B, T_, U), dtype=np.float32)
    for c in range(NCORES):
        bg, ug = divmod(c, NUG)
        oT = res.results[c]["outT"]  # [UHALF, BL*T_] holding v = 2h
        out[bg * BL : (bg + 1) * BL, :, ug * UHALF : (ug + 1) * UHALF] = (
            oT.reshape(UHALF, BL, T_).transpose(1, 2, 0)
        )
    out *= np.float32(0.5)
    return out



# revision 48
# speedup vs baseline: 1.2800x; 1.2703x over previous
"""BRU (bistable recurrent unit) cell kernel for 8 Trainium2 NeuronCores.

Hardcoded problem: B=64, T=512, D=1024, U=1024, fp32.

Sharding: 8 cores = 4 batch-groups (16 batches each) x 2 unit-groups
(512 units each).  Per core the three input projections
    projT[u, token] = K[d,u].T @ xT[d, token],   token = b*512 + t
run on the PE as a 1.5-pass split:
    x @ K  =  A@K1  +  (e*2^4)@(K1*2^-4) + (A*2^-8)@(K2*2^8)
with A = fp16(x) (exact residual e = x - A in fp32) and K1 = fp16(K),
K2 = K - K1.  The main term is one fp16 matmul (1 cycle/row); the two
correction products run as ONE fp8e5 DoubleRow matmul (0.5 cycles/row,
both slot products summed in-PE), accumulating into the same fp32 PSUM
group.  Power-of-two slot scalings cancel exactly, so each slot product
lands unscaled; fp8 rounding only perturbs the (already ~2^-11) residual
terms, leaving ~1e-4 projection error at 1.5x one pass's PE cost.

The 512-step recurrence is elementwise with u on partitions, split into
two batch-group chains that are software-pipelined by emission order
(engines dispatch in-order).  Sigmoid is re-expressed via tanh so each
group needs only 2 activations per step (one merged t1/tau tanh + one
hh tanh), and 7 fused DVE ops.  Projections are chunked TC steps at a
time (double-buffered) so the PE runs ahead of the recurrence;
PSUM->SBUF copies ride on the Scalar engine, dripped between scan
steps, and fold in the bias when nonzero.
"""

import os

import numpy as np

B, T, D, U = 64, 512, 1024, 1024
NCORES = 8
NBG = 4  # batch groups
NUG = 2  # unit groups
BL = B // NBG  # 16 batches per core
UHALF = U // NUG  # 512 units per core
UH = UHALF // 128  # 4 u-chunks

_CACHE: dict = {}


def _build(T_, TC, use_memory, use_bias):
    """Build and compile the per-core Bass program."""
    import concourse.mybir as mybir
    from concourse import bacc
    from concourse.tile import TileContext

    f32 = mybir.dt.float32
    f16 = mybir.dt.float16
    f8 = mybir.dt.float8e5
    Alu = mybir.AluOpType
    Act = mybir.ActivationFunctionType
    DR = mybir.MatmulPerfMode.DoubleRow

    NTOK = BL * T_
    NCH = T_ // TC
    DC = D // 128  # 8 d-chunks

    nc = bacc.Bacc("TRN2", target_bir_lowering=False, debug=False)

    xA = nc.dram_tensor("xA", [D, NTOK], f16, kind="ExternalInput").ap()
    x8d = nc.dram_tensor("x8d", [2, D, NTOK], f8, kind="ExternalInput").ap()
    k1 = {}
    k8 = {}
    for g in "zrh":
        k1[g] = nc.dram_tensor(f"k1{g}", [D, UHALF], f16, kind="ExternalInput").ap()
        k8[g] = nc.dram_tensor(f"k8{g}", [2, D, UHALF], f8, kind="ExternalInput").ap()
    if use_memory:
        mzb = nc.dram_tensor("mzb", [128, UH, BL], f32, kind="ExternalInput").ap()
        mrb = nc.dram_tensor("mrb", [128, UH, BL], f32, kind="ExternalInput").ap()
    if use_bias:
        bts = {
            g: nc.dram_tensor(f"bt{g}", [128, UH], f32, kind="ExternalInput").ap()
            for g in "zrh"
        }
    outT = nc.dram_tensor("outT", [UHALF, NTOK], f32, kind="ExternalOutput").ap()

    xA_r = xA.rearrange("(dc p) (b t) -> dc p b t", dc=DC, b=BL)
    x8_r = x8d.rearrange("two (dc p) (b t) -> two dc p b t", dc=DC, b=BL)
    outT_r = outT.rearrange("(uh p) (b t) -> uh p b t", uh=UH, b=BL)

    # Chunk schedule: optional short prefix chunks let the scan's first
    # rounds start as soon as a few projected columns exist instead of
    # waiting out a full TC-column matmul block.
    prefix = [
        int(p) for p in os.environ.get("BRU_PREFIX", "").split("+") if p
    ]
    assert all(0 < p <= TC for p in prefix), prefix
    rest = T_ - sum(prefix)
    assert rest % TC == 0, (prefix, T_)
    chunks = prefix + [TC] * (rest // TC)
    assert sum(chunks) == T_, (chunks, T_)

    with TileContext(nc) as tc:
        with (
            tc.tile_pool(name="weights", bufs=1) as wpool,
            tc.tile_pool(name="xin", bufs=2) as xpool,
            tc.tile_pool(name="proj", bufs=2) as ppool,
            tc.tile_pool(name="hout", bufs=3) as hpool,
            tc.tile_pool(name="tmp", bufs=12) as spool,
            tc.tile_pool(name="misc", bufs=1) as mpool,
            tc.tile_pool(name="psum", bufs=8, space="PSUM") as qpool,
        ):
            # Startup order: z-gate weights, then the first x chunk, then
            # the remaining weights, so the PE's first PSUM group can start
            # as early as possible.
            TC0 = chunks[0]
            w1 = {}
            w8 = {}
            for g in "zrh":
                w1[g] = wpool.tile([128, DC, UHALF], f16, tag=f"w1{g}", name=f"w1{g}")
                w8[g] = wpool.tile(
                    [128, 2, DC, UHALF], f8, tag=f"w8{g}", name=f"w8{g}"
                )
            nc.sync.dma_start(
                w1["z"][:, :, :], k1["z"].rearrange("(dc p) u -> p dc u", p=128)
            )
            xa = xpool.tile([128, DC, BL, TC], f16, tag="xa", name="xa_0")
            x8 = xpool.tile([128, 2, DC, BL, TC], f8, tag="x8", name="x8_0")
            for dc in range(DC):
                nc.sync.dma_start(xa[:, dc, :, :TC0], xA_r[dc, :, :, 0:TC0])
            for s in range(2):
                nc.sync.dma_start(
                    w8["z"][:, s, :, :],
                    k8["z"][s].rearrange("(dc p) u -> p dc u", p=128),
                )
            for dc in range(DC):
                for s in range(2):
                    nc.sync.dma_start(
                        x8[:, s, dc, :, :TC0], x8_r[s, dc, :, :, 0:TC0]
                    )
            first_x = (xa, x8)
            for g in "rh":
                nc.sync.dma_start(
                    w1[g][:, :, :], k1[g].rearrange("(dc p) u -> p dc u", p=128)
                )
                for s in range(2):
                    nc.sync.dma_start(
                        w8[g][:, s, :, :],
                        k8[g][s].rearrange("(dc p) u -> p dc u", p=128),
                    )
            if use_memory:
                # host passes mzb = 0.25*m_z, mrb = 0.5*m_r broadcasts
                mz4_t = mpool.tile([128, UH, BL], f32, tag="mz4", name="mz4")
                mr2_t = mpool.tile([128, UH, BL], f32, tag="mr2", name="mr2")
                nc.sync.dma_start(mz4_t[:, :, :], mzb[:, :, :])
                nc.sync.dma_start(mr2_t[:, :, :], mrb[:, :, :])
            if use_bias:
                b_t = {}
                for g in "zrh":
                    b_t[g] = mpool.tile([128, UH], f32, tag=f"b{g}", name=f"b{g}")
                    nc.sync.dma_start(b_t[g][:, :], bts[g][:, :])

            # ---------------------------------------------------------
            # Software-pipelined scan over two batch-group chains.
            #
            # Math (per step, with carried state v = 2h):
            #   t1  = tanh(h*m_r + xr)
            #   tau = tanh(0.5*(h*m_z + xz)) so  1-z = 0.5*(1-tau)
            #   hh  = tanh(xh + (t1+1)*h)
            #   v'  = 2h' = (v/2 + hh) + tau*(v/2 - hh)
            # The host folds 0.5 into the z-gate weights/bias and halves the
            # output, so the kernel stores v.
            #
            # The wall-clock of the scan is 512 x the per-chain serial step
            # latency (chains are batch splits; each runs all T steps), so
            # the emission is latency-driven: t1 and tau get SEPARATE Tanh
            # instructions so that only t1 sits on the serial path
            # v' -> t1in -> t1 -> w -> hin -> hh -> r1 -> v'; tau's Tanh and
            # the w2 blend ride in the act/DVE slack between path ops.  The
            # Activation engine's SBUF-ack (~185ns) is charged to every
            # cross-engine consumer, so each removed act visit saves ~450ns
            # of path.
            # ---------------------------------------------------------
            NCHAIN = int(os.environ.get("BRU_NCHAIN", "2"))
            if NCHAIN == 3:
                HBS = [6, 5, 5]
            else:
                HBS = [BL // NCHAIN] * NCHAIN
            bnds = [0]
            for hb in HBS:
                bnds.append(bnds[-1] + hb)
            GROUPS = tuple((bnds[i], bnds[i + 1]) for i in range(NCHAIN))

            v0t = []
            for gi in range(NCHAIN):
                vg = mpool.tile([128, UH, HBS[gi]], f32, tag=f"v0{gi}",
                                name=f"v0{gi}")
                nc.gpsimd.memset(vg[:, :, :], 0.0)
                v0t.append(vg)

            def tmp(tag, gi, shape=None):
                return spool.tile(shape or [128, UH, HBS[gi]], f32,
                                  tag=f"{tag}{gi}", name=f"{tag}{gi}")

            XMODE = os.environ.get("BRU_XMODE", "split")
            R1POS = os.environ.get("BRU_R1POS", "late")
            ALT_LEAD = os.environ.get("BRU_ALT_LEAD", "0") == "1"
            TAFAST = os.environ.get("BRU_TAFAST", "1") == "1" and not use_memory
            TBOLD = os.environ.get("BRU_TBOLD", "1") == "1"

            # Manual schedule gates (tile_wait_until): the Tile scheduler's
            # internal cost model has no sem/ack latencies, so its greedy
            # order stalls under the real timeline (e.g. it queues the other
            # chain's r1 ahead of a ready ta).  Gates pin each scan op to a
            # hand-packed per-round template instead; they only constrain the
            # compile-time schedule, not the hardware program.
            GATES = os.environ.get("BRU_GATES", "1") == "1"
            CYC = float(os.environ.get("BRU_CYC", "2150"))
            OFF = float(os.environ.get("BRU_OFF", "450"))
            GT0 = float(os.environ.get("BRU_T0", "40000"))
            G_OP = {
                "t1": 0.0, "tau": 212.0, "hh": 887.0,
                "w": 430.0, "hin": 619.0, "w2": 713.0,
                "taw": 910.0, "tbw": float(os.environ.get("BRU_G_TBW", "1004")), "r1": 1317.0,
                "ta": 1506.0,
                "tb": 1694.0 if os.environ.get("BRU_TBOLD", "1") == "1"
                else 1600.0,
                "v": 1600.0 if os.environ.get("BRU_TBOLD", "1") == "1"
                else 1694.0,
                "copy": 1523.0,
            }
            import contextlib

            def gat(op, s, gi=0):
                if not GATES:
                    return contextlib.nullcontext()
                t = GT0 + s * CYC + gi * OFF + G_OP[op]
                return tc.tile_wait_until(t / 1e6)

            SCBASE = [0]
            GSTEP = [0]

            state = [dict() for _ in range(NCHAIN)]
            fin = [None] * NCHAIN

            def op_v(gi, hch_g, trel):
                # v' = 0.5*w2 - (tau-1)*hh closes step trel of chain gi
                s = fin[gi]
                with gat("v", SCBASE[0] + trel, gi):
                    nc.vector.scalar_tensor_tensor(
                        hch_g[:, :, :, trel], s["w2"][:, :, :], 0.5,
                        s["r1"][:, :, :], Alu.mult, Alu.subtract,
                    )

            def stage_F(gi, v, pz, pr, trel):
                """a = t1in = h*m_r + xr;  b = 0.5*zin = h*mz/2 + xz/2.
                (xz/2 is pre-folded into the z projection host-side.)"""
                s = state[gi] = {}
                b0, b1 = GROUPS[gi]
                xr_t = pr[:, :, b0:b1, trel]
                xzh_t = pz[:, :, b0:b1, trel]
                if XMODE == "merged":
                    stg = tmp("stg", gi, [128, 2, UH, HBS[gi]])
                    s["ta"] = stg[:, 0]
                    s["tb"] = stg[:, 1]
                    s["stg"] = stg
                elif XMODE == "htau":
                    # tb shares a tile with hin: ONE Tanh at H-time yields
                    # both hh and tau, cutting the act count to 4/round while
                    # tau (only needed by the post-H blend) stays off the
                    # t1 serial path.
                    hb = tmp("hb", gi, [128, 2, UH, HBS[gi]])
                    s["ta"] = tmp("ta", gi)[:, :, :]
                    s["tb"] = hb[:, 1]
                    s["hb"] = hb
                else:
                    s["ta"] = tmp("ta", gi)[:, :, :]
                    s["tb"] = tmp("tb", gi)[:, :, :]
                if use_memory:
                    hm_r = tmp("hmr", gi)
                    hm_z = tmp("hmz", gi)
                    nc.vector.tensor_mul(hm_r[:, :, :], v, mr2_t[:, :, b0:b1])
                    nc.vector.tensor_add(s["ta"], hm_r[:, :, :], xr_t)
                    nc.vector.tensor_mul(hm_z[:, :, :], v, mz4_t[:, :, b0:b1])
                    nc.vector.tensor_add(s["tb"], hm_z[:, :, :], xzh_t)
                else:
                    with gat("ta", GSTEP[0] - 1, gi):
                        nc.vector.scalar_tensor_tensor(
                            s["ta"], v, 0.5, xr_t, Alu.mult, Alu.add
                        )
                    with gat("tb", GSTEP[0] - 1, gi):
                        nc.vector.scalar_tensor_tensor(
                            s["tb"], v, 0.25, xzh_t, Alu.mult, Alu.add
                        )
                s["v"] = v

            def stage_X(gi):
                s = state[gi]
                if XMODE == "merged":
                    sto = tmp("sto", gi, [128, 2, UH, HBS[gi]])
                    nc.scalar.activation(sto[:, :, :, :], s["stg"][:, :, :, :],
                                         Act.Tanh)
                    s["t1"] = sto[:, 0]
                    s["tau"] = sto[:, 1]
                elif XMODE == "htau":
                    s["t1"] = tmp("t1", gi)[:, :, :]
                    nc.scalar.activation(s["t1"], s["ta"], Act.Tanh)
                else:
                    s["t1"] = tmp("t1", gi)[:, :, :]
                    s["tau"] = tmp("tau", gi)[:, :, :]
                    with gat("t1", GSTEP[0], gi):
                        nc.scalar.activation(s["t1"], s["ta"], Act.Tanh)
                    with gat("tau", GSTEP[0], gi):
                        nc.scalar.activation(s["tau"], s["tb"], Act.Tanh)

            def stage_M(gi, ph, trel):
                # w2 sits between w -> hin so the same-engine RAW sem lag of
                # w is hidden behind w2's execution
                s = state[gi]
                b0, b1 = GROUPS[gi]
                xh_t = ph[:, :, b0:b1, trel]
                w = tmp("w", gi)
                with gat("w", GSTEP[0], gi):
                    nc.vector.scalar_tensor_tensor(
                        w[:, :, :], s["t1"], 1.0, s["v"], Alu.add, Alu.mult
                    )
                if XMODE == "htau":
                    s["hin"] = s["hb"][:, 0]
                else:
                    s["w2"] = tmp("w2", gi)
                    with gat("w2", GSTEP[0], gi):
                        nc.vector.scalar_tensor_tensor(
                            s["w2"][:, :, :], s["tau"], 1.0, s["v"],
                            Alu.add, Alu.mult,
                        )
                    s["hin"] = tmp("hin", gi)[:, :, :]
                with gat("hin", GSTEP[0], gi):
                    nc.vector.scalar_tensor_tensor(
                        s["hin"], w[:, :, :], 0.5, xh_t, Alu.mult, Alu.add
                    )

            def stage_H(gi):
                s = state[gi]
                if XMODE == "htau":
                    so = tmp("so", gi, [128, 2, UH, HBS[gi]])
                    nc.scalar.activation(so[:, :, :, :], s["hb"][:, :, :, :],
                                         Act.Tanh)
                    s["hh"] = so[:, 0]
                    s["tau"] = so[:, 1]
                else:
                    s["hh"] = tmp("hh", gi)[:, :, :]
                    with gat("hh", GSTEP[0], gi):
                        nc.scalar.activation(s["hh"], s["hin"], Act.Tanh)

            def op_w2(gi):
                s = state[gi]
                s["w2"] = tmp("w2", gi)
                nc.vector.scalar_tensor_tensor(
                    s["w2"][:, :, :], s["tau"], 1.0, s["v"], Alu.add, Alu.mult
                )

            def op_r1(gi):
                s = state[gi]
                s["r1"] = tmp("r1", gi)
                with gat("r1", GSTEP[0] - 1, gi):
                    nc.vector.scalar_tensor_tensor(
                        s["r1"][:, :, :], s["tau"], 1.0, s["hh"],
                        Alu.subtract, Alu.mult,
                    )

            # --- TAFAST path: the next step's tanh inputs skip v'. ---
            # ta(t+1) = 0.5*v'(t) + xr = (0.25*w2 + xr) - 0.5*r1, and the
            # (0.25*w2 + xr) half ("taw") only needs w2 — available a whole
            # act-visit earlier than v' — so the serial tail after hh is just
            # r1 -> ta instead of r1 -> v' -> ta.  v' itself (the stored
            # output) drops off the serial path entirely.
            def op_taw(gi, pz, pr, trel):
                s = state[gi]
                b0, b1 = GROUPS[gi]
                s["taw"] = tmp("taw", gi)
                with gat("taw", GSTEP[0] - 1, gi):
                    nc.vector.scalar_tensor_tensor(
                        s["taw"][:, :, :], s["w2"][:, :, :], 0.25,
                        pr[:, :, b0:b1, trel], Alu.mult, Alu.add,
                    )
                if not TBOLD:
                    s["tbw"] = tmp("tbw", gi)
                    with gat("tbw", GSTEP[0] - 1, gi):
                        nc.vector.scalar_tensor_tensor(
                            s["tbw"][:, :, :], s["w2"][:, :, :], 0.125,
                            pz[:, :, b0:b1, trel], Alu.mult, Alu.add,
                        )

            def stage_F_fast(gi, vnext, pz=None, trel=None):
                # consumes prev step's r1 + taw/tbw; v for the w/w2 ops is
                # the hch slice written by the (off-path) op_v
                pv = fin[gi]
                s = state[gi] = {}
                s["ta"] = tmp("ta", gi)[:, :, :]
                s["tb"] = tmp("tb", gi)[:, :, :]
                with gat("ta", GSTEP[0] - 1, gi):
                    nc.vector.scalar_tensor_tensor(
                        s["ta"], pv["r1"][:, :, :], -0.5, pv["taw"][:, :, :],
                        Alu.mult, Alu.add,
                    )
                if TBOLD:
                    b0, b1 = GROUPS[gi]
                    with gat("tb", GSTEP[0] - 1, gi):
                        nc.vector.scalar_tensor_tensor(
                            s["tb"], vnext, 0.25, pz[:, :, b0:b1, trel],
                            Alu.mult, Alu.add,
                        )
                else:
                    with gat("tb", GSTEP[0] - 1, gi):
                        nc.vector.scalar_tensor_tensor(
                            s["tb"], pv["r1"][:, :, :], -0.25,
                            pv["tbw"][:, :, :], Alu.mult, Alu.add,
                        )
                s["v"] = vnext

            def emit_matmuls(c, TCc, xa, x8):
                projs = {}
                copies = []
                for g in "zrh":
                    pg = ppool.tile(
                        [128, UH, BL, TC], f32, tag=f"p{g}", name=f"p{g}_{c}"
                    )
                    projs[g] = pg
                    for uh in range(UH):
                        us = slice(uh * 128, (uh + 1) * 128)
                        ps = qpool.tile([128, BL, TC], f32, tag="ps")
                        for dc in range(DC):
                            nc.tensor.matmul(
                                ps[:, :, :TCc], w1[g][:, dc, us], xa[:, dc, :, :TCc],
                                start=(dc == 0), stop=False,
                            )
                        for dc in range(DC):
                            nc.tensor.matmul(
                                ps[:, :, :TCc], w8[g][:, :, dc, us],
                                x8[:, :, dc, :, :TCc],
                                start=False, stop=(dc == DC - 1), perf_mode=DR,
                            )

                        copy_eng = os.environ.get("BRU_COPY_ENG", "act")
                        csplit = int(os.environ.get("BRU_COPY_SPLIT", "2"))
                        if csplit == 3:
                            bcuts = [0, 6, 11, 16]
                        else:
                            bcuts = [BL // csplit * j for j in range(csplit + 1)]

                        def mkcopy(pg=pg, uh=uh, ps=ps, g=g, ci=len(copies),
                                   copy_eng=copy_eng, b0=0, b1=BL):
                            def do():
                                if use_bias:
                                    # bias fold needs the act engine
                                    nc.scalar.activation(
                                        pg[:, uh, b0:b1, :TCc],
                                        ps[:, b0:b1, :TCc],
                                        Act.Identity, bias=b_t[g][:, uh : uh + 1],
                                    )
                                elif copy_eng == "act" or (
                                    copy_eng == "alt" and ci % 2 == 0
                                ):
                                    # Pool can't read PSUM and DMA can't source
                                    # it, so the drains go on the two queues
                                    # that can reach PSUM.  Split into pieces
                                    # small enough to fit the act engine's
                                    # per-round idle window.
                                    nc.scalar.activation(
                                        pg[:, uh, b0:b1, :TCc],
                                        ps[:, b0:b1, :TCc],
                                        Act.Identity,
                                    )
                                else:
                                    nc.vector.tensor_copy(
                                        pg[:, uh, b0:b1, :TCc],
                                        ps[:, b0:b1, :TCc],
                                    )
                            return do

                        for j in range(csplit):
                            copies.append(
                                mkcopy(b0=bcuts[j], b1=bcuts[j + 1])
                            )
                return projs, copies

            def emit_scan(sc, TCsc, projs, prev_v, prev_tc, pending):
                pz, pr, ph = projs["z"], projs["r"], projs["h"]
                hch = [
                    hpool.tile([128, UH, HBS[gi], TC], f32, tag=f"hch{gi}",
                               name=f"hch{gi}_{sc}")
                    for gi in range(NCHAIN)
                ]

                def v_of(gi, trel):
                    if trel == 0:
                        if sc == 0:
                            return v0t[gi][:, :, :]
                        return prev_v[gi][:, :, :, prev_tc - 1]
                    return hch[gi][:, :, :, trel - 1]

                ncopies = len(pending)
                emitted = 0

                def drip(trel):
                    nonlocal emitted
                    want = ((trel + 1) * ncopies) // max(TCsc - 1, 1)
                    while emitted < min(want, ncopies):
                        with gat("copy", SCBASE[0] + trel, 0):
                            pending[emitted]()
                        emitted += 1

                def round_body(trel, lead, emit_r1=True):
                    order = [(lead + i) % NCHAIN for i in range(NCHAIN)]
                    if trel > 0:
                        for g in order:
                            op_v(g, hch[g], trel - 1)
                    for g in order:
                        stage_F(g, v_of(g, trel), pz, pr, trel)
                        stage_X(g)
                    if R1POS == "inline" and emit_r1:
                        for g in order:
                            stage_M(g, ph, trel)
                            stage_H(g)
                            if XMODE == "htau":
                                op_w2(g)
                            op_r1(g)
                    else:
                        for g in order:
                            stage_M(g, ph, trel)
                            stage_H(g)
                        for g in order:
                            if XMODE == "htau":
                                op_w2(g)
                            if emit_r1:
                                op_r1(g)
                    for g in order:
                        fin[g] = state[g]

                def round_body_fast(trel, lead):
                    # steady-state round: r1/ta/tb of this step chain off the
                    # previous H directly; v' is written off-path
                    order = [(lead + i) % NCHAIN for i in range(NCHAIN)]
                    if trel == 0:
                        round_body(0, lead, emit_r1=False)
                        return
                    for g in order:
                        op_taw(g, pz, pr, trel)
                    for g in order:
                        op_r1(g)
                        if TBOLD:
                            op_v(g, hch[g], trel - 1)
                            stage_F_fast(g, v_of(g, trel), pz, trel)
                        else:
                            stage_F_fast(g, v_of(g, trel), pz, trel)
                            op_v(g, hch[g], trel - 1)
                        stage_X(g)
                    for g in order:
                        stage_M(g, ph, trel)
                        stage_H(g)
                    for g in order:
                        fin[g] = state[g]

                SCBASE[0] = t0s[sc]
                for trel in range(TCsc):
                    GSTEP[0] = SCBASE[0] + trel
                    lead = (trel % NCHAIN) if ALT_LEAD else 0
                    if TAFAST:
                        round_body_fast(trel, lead)
                    else:
                        round_body(trel, lead)
                    if trel > 0:
                        drip(trel - 1)
                GSTEP[0] = SCBASE[0] + TCsc
                if TAFAST:
                    for g in range(NCHAIN):
                        op_r1(g)
                for g in range(NCHAIN):
                    op_v(g, hch[g], TCsc - 1)
                while emitted < ncopies:
                    pending[emitted]()
                    emitted += 1
                return hch

            # main pipeline over chunks
            prev_v = None
            prev_tc = None
            prev_projs = None
            t0 = 0
            t0s = []
            for c, TCc in enumerate(chunks):
                if c == 0:
                    xa, x8 = first_x
                else:
                    xa = xpool.tile([128, DC, BL, TC], f16, tag="xa", name=f"xa_{c}")
                    x8 = xpool.tile(
                        [128, 2, DC, BL, TC], f8, tag="x8", name=f"x8_{c}"
                    )
                    for dc in range(DC):
                        nc.sync.dma_start(
                            xa[:, dc, :, :TCc], xA_r[dc, :, :, t0 : t0 + TCc]
                        )
                        for s in range(2):
                            nc.sync.dma_start(
                                x8[:, s, dc, :, :TCc],
                                x8_r[s, dc, :, :, t0 : t0 + TCc],
                            )
                projs, copies = emit_matmuls(c, TCc, xa, x8)
                if c == 0:
                    for do in copies:
                        do()
                else:
                    sc = c - 1
                    TCsc = chunks[sc]
                    hch = emit_scan(sc, TCsc, prev_projs, prev_v, prev_tc, copies)
                    for uh in range(UH):
                        for gi, (b0, b1) in enumerate(GROUPS):
                            nc.sync.dma_start(
                                outT_r[uh, :, b0:b1, t0s[sc] : t0s[sc] + TCsc],
                                hch[gi][:, uh, :, :TCsc],
                            )
                    prev_v = hch
                    prev_tc = TCsc
                prev_projs = projs
                t0s.append(t0)
                t0 += TCc
            sc = len(chunks) - 1
            TCsc = chunks[sc]
            hch = emit_scan(sc, TCsc, prev_projs, prev_v, prev_tc, [])
            # quarter the final chunk's writeback so it streams out behind
            # the scan instead of serializing after the last step
            QS = max(TCsc // 4, 1)
            for q0 in range(0, TCsc, QS):
                q1 = min(q0 + QS, TCsc)
                for uh in range(UH):
                    for gi, (b0, b1) in enumerate(GROUPS):
                        nc.sync.dma_start(
                            outT_r[uh, :, b0:b1, t0s[sc] + q0 : t0s[sc] + q1],
                            hch[gi][:, uh, :, q0:q1],
                        )

    nc.compile()
    return nc


def _get_nc(T_, TC, use_memory, use_bias):
    key = (T_, TC, use_memory, use_bias)
    if key not in _CACHE:
        _CACHE[key] = _build(T_, TC, use_memory, use_bias)
    return _CACHE[key]


def kernel(
    x,
    kernel_z,
    kernel_r,
    kernel_h,
    memory_z,
    memory_r,
    bias_z,
    bias_r,
    bias_h,
):
    from concourse import bass_utils

    x = np.asarray(x, dtype=np.float32)
    Ks = {
        "z": np.asarray(kernel_z, dtype=np.float32),
        "r": np.asarray(kernel_r, dtype=np.float32),
        "h": np.asarray(kernel_h, dtype=np.float32),
    }
    mem = {
        "z": np.asarray(memory_z, dtype=np.float32),
        "r": np.asarray(memory_r, dtype=np.float32),
    }
    bias = {
        "z": np.asarray(bias_z, dtype=np.float32),
        "r": np.asarray(bias_r, dtype=np.float32),
        "h": np.asarray(bias_h, dtype=np.float32),
    }

    B_, T_, D_ = x.shape
    assert (B_, D_) == (B, D), (x.shape,)
    TC = int(os.environ.get("BRU_TC", "32"))

    use_memory = not all(np.all(m == 1.0) for m in mem.values())
    use_bias = not all(np.all(b == 0.0) for b in bias.values())

    nc = _get_nc(T_, TC, use_memory, use_bias)

    import ml_dtypes

    f8e5 = ml_dtypes.float8_e5m2

    # Split weights once (shared across cores).  The z-gate weights/bias are
    # pre-halved: the kernel computes tau = tanh(0.5*zin) instead of
    # sigmoid(zin).  Each gate ships the fp16 main K1 plus a DoubleRow fp8
    # pair [K1*2^-4, K2*2^8] whose slot scalings cancel against the fp8
    # moving pair [e*2^4, A*2^-8].
    w1_full = {}
    w8_full = {}
    for g, K in Ks.items():
        if g == "z":
            K = K * np.float32(0.5)
        K1 = K.astype(np.float16)
        K2 = K - K1.astype(np.float32)
        k8 = np.empty((2, D, K.shape[1]), dtype=f8e5)
        k8[0] = (K1.astype(np.float32) * np.float32(2.0 ** -4)).astype(f8e5)
        k8[1] = (K2 * np.float32(2.0 ** 8)).astype(f8e5)
        w1_full[g] = K1
        w8_full[g] = k8

    in_maps = []
    for c in range(NCORES):
        bg, ug = divmod(c, NUG)
        xc = x[bg * BL : (bg + 1) * BL].reshape(BL * T_, D)
        xcT = np.ascontiguousarray(xc.T)  # [D, NTOK] fp32
        A = xcT.astype(np.float16)
        e = xcT - A.astype(np.float32)
        x8 = np.empty((2, D, xcT.shape[1]), dtype=f8e5)
        x8[0] = (e * np.float32(16.0)).astype(f8e5)
        x8[1] = (A.astype(np.float32) * np.float32(2.0 ** -8)).astype(f8e5)
        us = slice(ug * UHALF, (ug + 1) * UHALF)
        m = {"xA": A, "x8d": x8}
        for g in "zrh":
            m[f"k1{g}"] = np.ascontiguousarray(w1_full[g][:, us])
            m[f"k8{g}"] = np.ascontiguousarray(w8_full[g][:, :, us])
        if use_memory:
            # element (p, uh, b) = mem[ug*UHALF + uh*128 + p], pre-scaled
            for name, v, sc_ in (
                ("mzb", mem["z"], 0.25),
                ("mrb", mem["r"], 0.5),
            ):
                mv = (v[us] * np.float32(sc_)).reshape(UH, 128).T  # [128, UH]
                m[name] = np.ascontiguousarray(
                    np.broadcast_to(mv[:, :, None], (128, UH, BL))
                )
        if use_bias:
            for g in "zrh":
                bv = bias[g][us]
                if g == "z":
                    bv = bv * np.float32(0.5)
                m[f"bt{g}"] = np.ascontiguousarray(bv.reshape(UH, 128).T)
        in_maps.append(m)

    res = bass_utils.run_bass_kernel_spmd(nc, in_maps, core_ids=list(range(NCORES)))

    out = np.empty((B, T_, U), dtype=np.float32)
    for c in range(NCORES):
        bg, ug = divmod(c, NUG)
        oT = res.results[c]["outT"]  # [UHALF, BL*T_] holding v = 2h
        out[bg * BL : (bg + 1) * BL, :, ug * UHALF : (ug + 1) * UHALF] = (
            oT.reshape(UHALF, BL, T_).transpose(1, 2, 0)
        )
    out *= np.float32(0.5)
    return out



# revision 50
# speedup vs baseline: 1.2833x; 1.0026x over previous
"""BRU (bistable recurrent unit) cell kernel for 8 Trainium2 NeuronCores.

Hardcoded problem: B=64, T=512, D=1024, U=1024, fp32.

Sharding: 8 cores = 4 batch-groups (16 batches each) x 2 unit-groups
(512 units each).  Per core the three input projections
    projT[u, token] = K[d,u].T @ xT[d, token],   token = b*512 + t
run on the PE as a 1.5-pass split:
    x @ K  =  A@K1  +  (e*2^4)@(K1*2^-4) + (A*2^-8)@(K2*2^8)
with A = fp16(x) (exact residual e = x - A in fp32) and K1 = fp16(K),
K2 = K - K1.  The main term is one fp16 matmul (1 cycle/row); the two
correction products run as ONE fp8e5 DoubleRow matmul (0.5 cycles/row,
both slot products summed in-PE), accumulating into the same fp32 PSUM
group.  Power-of-two slot scalings cancel exactly, so each slot product
lands unscaled; fp8 rounding only perturbs the (already ~2^-11) residual
terms, leaving ~1e-4 projection error at 1.5x one pass's PE cost.

The 512-step recurrence is elementwise with u on partitions, split into
two batch-group chains.  The wall-clock is 512 x the per-chain serial
step latency, so the scan is latency-engineered end to end:

- Sigmoid is re-expressed via tau = tanh(0.5 zin); t1 and tau get
  SEPARATE Tanh instructions so only t1 sits on the serial path.
- The next step's tanh inputs skip v' ("TAFAST"): with
  v' = 0.5 w2 - r1, ta(t+1) = (0.25 w2 + xr) - 0.5 r1, where the w2
  half precomputes off-path a whole act-visit early, leaving only
  r1 -> ta on the tail; v' itself (the stored output) is written
  off-path.  tb(t+1) reads the freshly written v' directly ("TBOLD").
- The Tile scheduler's internal cost model has no semaphore/ack
  latencies (the Act engine's SBUF-ack alone is ~185ns charged to every
  cross-engine consumer), so its greedy order stalls under the real
  timeline.  tile_wait_until gates pin every scan op to a hand-packed
  per-round template (CYC ns per step, chain 1 offset by OFF) that
  keeps both DVE (16 ops/round) and Act (6 tanh/round) ~90% busy with
  the serial path threading through the gaps.  Gates only shape the
  compile-time schedule; the emitted program carries no extra waits.
- PSUM->SBUF projection copies are split in half so each piece fits the
  Act engine's per-round idle window, and the last chunk's writeback is
  quartered so it streams out behind the scan.

Steady-state round: ~1.76us for 2 chains x 1 step (vs ~2.4us for the
naive emission); total ~1.00ms vs the 1.21ms 3-pass baseline.
"""

import os

import numpy as np

B, T, D, U = 64, 512, 1024, 1024
NCORES = 8
NBG = 4  # batch groups
NUG = 2  # unit groups
BL = B // NBG  # 16 batches per core
UHALF = U // NUG  # 512 units per core
UH = UHALF // 128  # 4 u-chunks

_CACHE: dict = {}


def _build(T_, TC, use_memory, use_bias):
    """Build and compile the per-core Bass program."""
    import concourse.mybir as mybir
    from concourse import bacc
    from concourse.tile import TileContext

    f32 = mybir.dt.float32
    f16 = mybir.dt.float16
    f8 = mybir.dt.float8e5
    Alu = mybir.AluOpType
    Act = mybir.ActivationFunctionType
    DR = mybir.MatmulPerfMode.DoubleRow

    NTOK = BL * T_
    NCH = T_ // TC
    DC = D // 128  # 8 d-chunks

    nc = bacc.Bacc("TRN2", target_bir_lowering=False, debug=False)

    xA = nc.dram_tensor("xA", [D, NTOK], f16, kind="ExternalInput").ap()
    x8d = nc.dram_tensor("x8d", [2, D, NTOK], f8, kind="ExternalInput").ap()
    k1 = {}
    k8 = {}
    for g in "zrh":
        k1[g] = nc.dram_tensor(f"k1{g}", [D, UHALF], f16, kind="ExternalInput").ap()
        k8[g] = nc.dram_tensor(f"k8{g}", [2, D, UHALF], f8, kind="ExternalInput").ap()
    if use_memory:
        mzb = nc.dram_tensor("mzb", [128, UH, BL], f32, kind="ExternalInput").ap()
        mrb = nc.dram_tensor("mrb", [128, UH, BL], f32, kind="ExternalInput").ap()
    if use_bias:
        bts = {
            g: nc.dram_tensor(f"bt{g}", [128, UH], f32, kind="ExternalInput").ap()
            for g in "zrh"
        }
    outT = nc.dram_tensor("outT", [UHALF, NTOK], f32, kind="ExternalOutput").ap()

    xA_r = xA.rearrange("(dc p) (b t) -> dc p b t", dc=DC, b=BL)
    x8_r = x8d.rearrange("two (dc p) (b t) -> two dc p b t", dc=DC, b=BL)
    outT_r = outT.rearrange("(uh p) (b t) -> uh p b t", uh=UH, b=BL)

    # Chunk schedule: optional short prefix chunks let the scan's first
    # rounds start as soon as a few projected columns exist instead of
    # waiting out a full TC-column matmul block.
    prefix = [
        int(p) for p in os.environ.get("BRU_PREFIX", "").split("+") if p
    ]
    assert all(0 < p <= TC for p in prefix), prefix
    rest = T_ - sum(prefix)
    assert rest % TC == 0, (prefix, T_)
    chunks = prefix + [TC] * (rest // TC)
    assert sum(chunks) == T_, (chunks, T_)

    with TileContext(nc) as tc:
        with (
            tc.tile_pool(name="weights", bufs=1) as wpool,
            tc.tile_pool(name="xin", bufs=2) as xpool,
            tc.tile_pool(name="proj", bufs=2) as ppool,
            tc.tile_pool(name="hout", bufs=3) as hpool,
            tc.tile_pool(name="tmp", bufs=12) as spool,
            tc.tile_pool(name="misc", bufs=1) as mpool,
            tc.tile_pool(name="psum", bufs=8, space="PSUM") as qpool,
        ):
            # Startup order: z-gate weights, then the first x chunk, then
            # the remaining weights, so the PE's first PSUM group can start
            # as early as possible.
            TC0 = chunks[0]
            w1 = {}
            w8 = {}
            for g in "zrh":
                w1[g] = wpool.tile([128, DC, UHALF], f16, tag=f"w1{g}", name=f"w1{g}")
                w8[g] = wpool.tile(
                    [128, 2, DC, UHALF], f8, tag=f"w8{g}", name=f"w8{g}"
                )
            nc.sync.dma_start(
                w1["z"][:, :, :], k1["z"].rearrange("(dc p) u -> p dc u", p=128)
            )
            xa = xpool.tile([128, DC, BL, TC], f16, tag="xa", name="xa_0")
            x8 = xpool.tile([128, 2, DC, BL, TC], f8, tag="x8", name="x8_0")
            for dc in range(DC):
                nc.sync.dma_start(xa[:, dc, :, :TC0], xA_r[dc, :, :, 0:TC0])
            for s in range(2):
                nc.sync.dma_start(
                    w8["z"][:, s, :, :],
                    k8["z"][s].rearrange("(dc p) u -> p dc u", p=128),
                )
            for dc in range(DC):
                for s in range(2):
                    nc.sync.dma_start(
                        x8[:, s, dc, :, :TC0], x8_r[s, dc, :, :, 0:TC0]
                    )
            first_x = (xa, x8)
            for g in "rh":
                nc.sync.dma_start(
                    w1[g][:, :, :], k1[g].rearrange("(dc p) u -> p dc u", p=128)
                )
                for s in range(2):
                    nc.sync.dma_start(
                        w8[g][:, s, :, :],
                        k8[g][s].rearrange("(dc p) u -> p dc u", p=128),
                    )
            if use_memory:
                # host passes mzb = 0.25*m_z, mrb = 0.5*m_r broadcasts
                mz4_t = mpool.tile([128, UH, BL], f32, tag="mz4", name="mz4")
                mr2_t = mpool.tile([128, UH, BL], f32, tag="mr2", name="mr2")
                nc.sync.dma_start(mz4_t[:, :, :], mzb[:, :, :])
                nc.sync.dma_start(mr2_t[:, :, :], mrb[:, :, :])
            if use_bias:
                b_t = {}
                for g in "zrh":
                    b_t[g] = mpool.tile([128, UH], f32, tag=f"b{g}", name=f"b{g}")
                    nc.sync.dma_start(b_t[g][:, :], bts[g][:, :])

            # ---------------------------------------------------------
            # Software-pipelined scan over two batch-group chains.
            #
            # Math (per step, with carried state v = 2h):
            #   t1  = tanh(h*m_r + xr)
            #   tau = tanh(0.5*(h*m_z + xz)) so  1-z = 0.5*(1-tau)
            #   hh  = tanh(xh + (t1+1)*h)
            #   v'  = 2h' = (v/2 + hh) + tau*(v/2 - hh)
            # The host folds 0.5 into the z-gate weights/bias and halves the
            # output, so the kernel stores v.
            #
            # The wall-clock of the scan is 512 x the per-chain serial step
            # latency (chains are batch splits; each runs all T steps), so
            # the emission is latency-driven: t1 and tau get SEPARATE Tanh
            # instructions so that only t1 sits on the serial path
            # v' -> t1in -> t1 -> w -> hin -> hh -> r1 -> v'; tau's Tanh and
            # the w2 blend ride in the act/DVE slack between path ops.  The
            # Activation engine's SBUF-ack (~185ns) is charged to every
            # cross-engine consumer, so each removed act visit saves ~450ns
            # of path.
            # ---------------------------------------------------------
            NCHAIN = int(os.environ.get("BRU_NCHAIN", "2"))
            if NCHAIN == 3:
                HBS = [6, 5, 5]
            else:
                HBS = [BL // NCHAIN] * NCHAIN
            bnds = [0]
            for hb in HBS:
                bnds.append(bnds[-1] + hb)
            GROUPS = tuple((bnds[i], bnds[i + 1]) for i in range(NCHAIN))

            v0t = []
            for gi in range(NCHAIN):
                vg = mpool.tile([128, UH, HBS[gi]], f32, tag=f"v0{gi}",
                                name=f"v0{gi}")
                nc.gpsimd.memset(vg[:, :, :], 0.0)
                v0t.append(vg)

            def tmp(tag, gi, shape=None):
                return spool.tile(shape or [128, UH, HBS[gi]], f32,
                                  tag=f"{tag}{gi}", name=f"{tag}{gi}")

            XMODE = os.environ.get("BRU_XMODE", "split")
            R1POS = os.environ.get("BRU_R1POS", "late")
            ALT_LEAD = os.environ.get("BRU_ALT_LEAD", "0") == "1"
            TAFAST = os.environ.get("BRU_TAFAST", "1") == "1" and not use_memory
            TBOLD = os.environ.get("BRU_TBOLD", "1") == "1"

            # Manual schedule gates (tile_wait_until): the Tile scheduler's
            # internal cost model has no sem/ack latencies, so its greedy
            # order stalls under the real timeline (e.g. it queues the other
            # chain's r1 ahead of a ready ta).  Gates pin each scan op to a
            # hand-packed per-round template instead; they only constrain the
            # compile-time schedule, not the hardware program.
            GATES = os.environ.get("BRU_GATES", "1") == "1"
            CYC = float(os.environ.get("BRU_CYC", "3500"))
            OFF = float(os.environ.get("BRU_OFF", "480"))
            GT0 = float(os.environ.get("BRU_T0", "40000"))
            G_OP = {
                "t1": 0.0, "tau": 212.0, "hh": 887.0,
                "w": 430.0, "hin": 619.0, "w2": 713.0,
                "taw": 910.0, "tbw": float(os.environ.get("BRU_G_TBW", "1004")), "r1": 1317.0,
                "ta": 1506.0,
                "tb": 1694.0 if os.environ.get("BRU_TBOLD", "1") == "1"
                else 1600.0,
                "v": 1600.0 if os.environ.get("BRU_TBOLD", "1") == "1"
                else 1694.0,
                "copy": 1523.0,
            }
            for kv in os.environ.get("BRU_GOP", "").split("+"):
                if kv:
                    k, _, v = kv.partition("=")
                    G_OP[k] = float(v)
            import contextlib

            def gat(op, s, gi=0):
                if not GATES:
                    return contextlib.nullcontext()
                t = GT0 + s * CYC + gi * OFF + G_OP[op]
                return tc.tile_wait_until(t / 1e6)

            SCBASE = [0]
            GSTEP = [0]

            state = [dict() for _ in range(NCHAIN)]
            fin = [None] * NCHAIN

            def op_v(gi, hch_g, trel):
                # v' = 0.5*w2 - (tau-1)*hh closes step trel of chain gi
                s = fin[gi]
                with gat("v", SCBASE[0] + trel, gi):
                    nc.vector.scalar_tensor_tensor(
                        hch_g[:, :, :, trel], s["w2"][:, :, :], 0.5,
                        s["r1"][:, :, :], Alu.mult, Alu.subtract,
                    )

            def stage_F(gi, v, pz, pr, trel):
                """a = t1in = h*m_r + xr;  b = 0.5*zin = h*mz/2 + xz/2.
                (xz/2 is pre-folded into the z projection host-side.)"""
                s = state[gi] = {}
                b0, b1 = GROUPS[gi]
                xr_t = pr[:, :, b0:b1, trel]
                xzh_t = pz[:, :, b0:b1, trel]
                if XMODE == "merged":
                    stg = tmp("stg", gi, [128, 2, UH, HBS[gi]])
                    s["ta"] = stg[:, 0]
                    s["tb"] = stg[:, 1]
                    s["stg"] = stg
                elif XMODE == "htau":
                    # tb shares a tile with hin: ONE Tanh at H-time yields
                    # both hh and tau, cutting the act count to 4/round while
                    # tau (only needed by the post-H blend) stays off the
                    # t1 serial path.
                    hb = tmp("hb", gi, [128, 2, UH, HBS[gi]])
                    s["ta"] = tmp("ta", gi)[:, :, :]
                    s["tb"] = hb[:, 1]
                    s["hb"] = hb
                else:
                    s["ta"] = tmp("ta", gi)[:, :, :]
                    s["tb"] = tmp("tb", gi)[:, :, :]
                if use_memory:
                    hm_r = tmp("hmr", gi)
                    hm_z = tmp("hmz", gi)
                    nc.vector.tensor_mul(hm_r[:, :, :], v, mr2_t[:, :, b0:b1])
                    nc.vector.tensor_add(s["ta"], hm_r[:, :, :], xr_t)
                    nc.vector.tensor_mul(hm_z[:, :, :], v, mz4_t[:, :, b0:b1])
                    nc.vector.tensor_add(s["tb"], hm_z[:, :, :], xzh_t)
                else:
                    with gat("ta", GSTEP[0] - 1, gi):
                        nc.vector.scalar_tensor_tensor(
                            s["ta"], v, 0.5, xr_t, Alu.mult, Alu.add
                        )
                    with gat("tb", GSTEP[0] - 1, gi):
                        nc.vector.scalar_tensor_tensor(
                            s["tb"], v, 0.25, xzh_t, Alu.mult, Alu.add
                        )
                s["v"] = v

            def stage_X(gi):
                s = state[gi]
                if XMODE == "merged":
                    sto = tmp("sto", gi, [128, 2, UH, HBS[gi]])
                    nc.scalar.activation(sto[:, :, :, :], s["stg"][:, :, :, :],
                                         Act.Tanh)
                    s["t1"] = sto[:, 0]
                    s["tau"] = sto[:, 1]
                elif XMODE == "htau":
                    s["t1"] = tmp("t1", gi)[:, :, :]
                    nc.scalar.activation(s["t1"], s["ta"], Act.Tanh)
                else:
                    s["t1"] = tmp("t1", gi)[:, :, :]
                    s["tau"] = tmp("tau", gi)[:, :, :]
                    with gat("t1", GSTEP[0], gi):
                        nc.scalar.activation(s["t1"], s["ta"], Act.Tanh)
                    with gat("tau", GSTEP[0], gi):
                        nc.scalar.activation(s["tau"], s["tb"], Act.Tanh)

            def stage_M(gi, ph, trel):
                # w2 sits between w -> hin so the same-engine RAW sem lag of
                # w is hidden behind w2's execution
                s = state[gi]
                b0, b1 = GROUPS[gi]
                xh_t = ph[:, :, b0:b1, trel]
                w = tmp("w", gi)
                with gat("w", GSTEP[0], gi):
                    nc.vector.scalar_tensor_tensor(
                        w[:, :, :], s["t1"], 1.0, s["v"], Alu.add, Alu.mult
                    )
                if XMODE == "htau":
                    s["hin"] = s["hb"][:, 0]
                else:
                    s["w2"] = tmp("w2", gi)
                    with gat("w2", GSTEP[0], gi):
                        nc.vector.scalar_tensor_tensor(
                            s["w2"][:, :, :], s["tau"], 1.0, s["v"],
                            Alu.add, Alu.mult,
                        )
                    s["hin"] = tmp("hin", gi)[:, :, :]
                with gat("hin", GSTEP[0], gi):
                    nc.vector.scalar_tensor_tensor(
                        s["hin"], w[:, :, :], 0.5, xh_t, Alu.mult, Alu.add
                    )

            def stage_H(gi):
                s = state[gi]
                if XMODE == "htau":
                    so = tmp("so", gi, [128, 2, UH, HBS[gi]])
                    nc.scalar.activation(so[:, :, :, :], s["hb"][:, :, :, :],
                                         Act.Tanh)
                    s["hh"] = so[:, 0]
                    s["tau"] = so[:, 1]
                else:
                    s["hh"] = tmp("hh", gi)[:, :, :]
                    with gat("hh", GSTEP[0], gi):
                        nc.scalar.activation(s["hh"], s["hin"], Act.Tanh)

            def op_w2(gi):
                s = state[gi]
                s["w2"] = tmp("w2", gi)
                nc.vector.scalar_tensor_tensor(
                    s["w2"][:, :, :], s["tau"], 1.0, s["v"], Alu.add, Alu.mult
                )

            def op_r1(gi):
                s = state[gi]
                s["r1"] = tmp("r1", gi)
                with gat("r1", GSTEP[0] - 1, gi):
                    nc.vector.scalar_tensor_tensor(
                        s["r1"][:, :, :], s["tau"], 1.0, s["hh"],
                        Alu.subtract, Alu.mult,
                    )

            # --- TAFAST path: the next step's tanh inputs skip v'. ---
            # ta(t+1) = 0.5*v'(t) + xr = (0.25*w2 + xr) - 0.5*r1, and the
            # (0.25*w2 + xr) half ("taw") only needs w2 — available a whole
            # act-visit earlier than v' — so the serial tail after hh is just
            # r1 -> ta instead of r1 -> v' -> ta.  v' itself (the stored
            # output) drops off the serial path entirely.
            def op_taw(gi, pz, pr, trel):
                s = state[gi]
                b0, b1 = GROUPS[gi]
                s["taw"] = tmp("taw", gi)
                with gat("taw", GSTEP[0] - 1, gi):
                    nc.vector.scalar_tensor_tensor(
                        s["taw"][:, :, :], s["w2"][:, :, :], 0.25,
                        pr[:, :, b0:b1, trel], Alu.mult, Alu.add,
                    )
                if not TBOLD:
                    s["tbw"] = tmp("tbw", gi)
                    with gat("tbw", GSTEP[0] - 1, gi):
                        nc.vector.scalar_tensor_tensor(
                            s["tbw"][:, :, :], s["w2"][:, :, :], 0.125,
                            pz[:, :, b0:b1, trel], Alu.mult, Alu.add,
                        )

            def stage_F_fast(gi, vnext, pz=None, trel=None):
                # consumes prev step's r1 + taw/tbw; v for the w/w2 ops is
                # the hch slice written by the (off-path) op_v
                pv = fin[gi]
                s = state[gi] = {}
                s["ta"] = tmp("ta", gi)[:, :, :]
                s["tb"] = tmp("tb", gi)[:, :, :]
                with gat("ta", GSTEP[0] - 1, gi):
                    nc.vector.scalar_tensor_tensor(
                        s["ta"], pv["r1"][:, :, :], -0.5, pv["taw"][:, :, :],
                        Alu.mult, Alu.add,
                    )
                if TBOLD:
                    b0, b1 = GROUPS[gi]
                    with gat("tb", GSTEP[0] - 1, gi):
                        nc.vector.scalar_tensor_tensor(
                            s["tb"], vnext, 0.25, pz[:, :, b0:b1, trel],
                            Alu.mult, Alu.add,
                        )
                else:
                    with gat("tb", GSTEP[0] - 1, gi):
                        nc.vector.scalar_tensor_tensor(
                            s["tb"], pv["r1"][:, :, :], -0.25,
                            pv["tbw"][:, :, :], Alu.mult, Alu.add,
                        )
                s["v"] = vnext

            def emit_matmuls(c, TCc, xa, x8):
                projs = {}
                copies = []
                for g in "zrh":
                    pg = ppool.tile(
                        [128, UH, BL, TC], f32, tag=f"p{g}", name=f"p{g}_{c}"
                    )
                    projs[g] = pg
                    for uh in range(UH):
                        us = slice(uh * 128, (uh + 1) * 128)
                        ps = qpool.tile([128, BL, TC], f32, tag="ps")
                        for dc in range(DC):
                            nc.tensor.matmul(
                                ps[:, :, :TCc], w1[g][:, dc, us], xa[:, dc, :, :TCc],
                                start=(dc == 0), stop=False,
                            )
                        for dc in range(DC):
                            nc.tensor.matmul(
                                ps[:, :, :TCc], w8[g][:, :, dc, us],
                                x8[:, :, dc, :, :TCc],
                                start=False, stop=(dc == DC - 1), perf_mode=DR,
                            )

                        copy_eng = os.environ.get("BRU_COPY_ENG", "act")
                        csplit = int(os.environ.get("BRU_COPY_SPLIT", "2"))
                        if csplit == 3:
                            bcuts = [0, 6, 11, 16]
                        else:
                            bcuts = [BL // csplit * j for j in range(csplit + 1)]

                        def mkcopy(pg=pg, uh=uh, ps=ps, g=g, ci=len(copies),
                                   copy_eng=copy_eng, b0=0, b1=BL):
                            def do():
                                if use_bias:
                                    # bias fold needs the act engine
                                    nc.scalar.activation(
                                        pg[:, uh, b0:b1, :TCc],
                                        ps[:, b0:b1, :TCc],
                                        Act.Identity, bias=b_t[g][:, uh : uh + 1],
                                    )
                                elif copy_eng == "act" or (
                                    copy_eng == "alt" and ci % 2 == 0
                                ):
                                    # Pool can't read PSUM and DMA can't source
                                    # it, so the drains go on the two queues
                                    # that can reach PSUM.  Split into pieces
                                    # small enough to fit the act engine's
                                    # per-round idle window.
                                    nc.scalar.activation(
                                        pg[:, uh, b0:b1, :TCc],
                                        ps[:, b0:b1, :TCc],
                                        Act.Identity,
                                    )
                                else:
                                    nc.vector.tensor_copy(
                                        pg[:, uh, b0:b1, :TCc],
                                        ps[:, b0:b1, :TCc],
                                    )
                            return do

                        for j in range(csplit):
                            copies.append(
                                mkcopy(b0=bcuts[j], b1=bcuts[j + 1])
                            )
                return projs, copies

            def emit_scan(sc, TCsc, projs, prev_v, prev_tc, pending):
                pz, pr, ph = projs["z"], projs["r"], projs["h"]
                hch = [
                    hpool.tile([128, UH, HBS[gi], TC], f32, tag=f"hch{gi}",
                               name=f"hch{gi}_{sc}")
                    for gi in range(NCHAIN)
                ]

                def v_of(gi, trel):
                    if trel == 0:
                        if sc == 0:
                            return v0t[gi][:, :, :]
                        return prev_v[gi][:, :, :, prev_tc - 1]
                    return hch[gi][:, :, :, trel - 1]

                ncopies = len(pending)
                emitted = 0

                def drip(trel):
                    nonlocal emitted
                    want = ((trel + 1) * ncopies) // max(TCsc - 1, 1)
                    while emitted < min(want, ncopies):
                        with gat("copy", SCBASE[0] + trel, 0):
                            pending[emitted]()
                        emitted += 1

                def round_body(trel, lead, emit_r1=True):
                    order = [(lead + i) % NCHAIN for i in range(NCHAIN)]
                    if trel > 0:
                        for g in order:
                            op_v(g, hch[g], trel - 1)
                    for g in order:
                        stage_F(g, v_of(g, trel), pz, pr, trel)
                        stage_X(g)
                    if R1POS == "inline" and emit_r1:
                        for g in order:
                            stage_M(g, ph, trel)
                            stage_H(g)
                            if XMODE == "htau":
                                op_w2(g)
                            op_r1(g)
                    else:
                        for g in order:
                            stage_M(g, ph, trel)
                            stage_H(g)
                        for g in order:
                            if XMODE == "htau":
                                op_w2(g)
                            if emit_r1:
                                op_r1(g)
                    for g in order:
                        fin[g] = state[g]

                def round_body_fast(trel, lead):
                    # steady-state round: r1/ta/tb of this step chain off the
                    # previous H directly; v' is written off-path
                    order = [(lead + i) % NCHAIN for i in range(NCHAIN)]
                    if trel == 0:
                        round_body(0, lead, emit_r1=False)
                        return
                    for g in order:
                        op_taw(g, pz, pr, trel)
                    for g in order:
                        op_r1(g)
                        if TBOLD:
                            op_v(g, hch[g], trel - 1)
                            stage_F_fast(g, v_of(g, trel), pz, trel)
                        else:
                            stage_F_fast(g, v_of(g, trel), pz, trel)
                            op_v(g, hch[g], trel - 1)
                        stage_X(g)
                    for g in order:
                        stage_M(g, ph, trel)
                        stage_H(g)
                    for g in order:
                        fin[g] = state[g]

                SCBASE[0] = t0s[sc]
                for trel in range(TCsc):
                    GSTEP[0] = SCBASE[0] + trel
                    lead = (trel % NCHAIN) if ALT_LEAD else 0
                    if TAFAST:
                        round_body_fast(trel, lead)
                    else:
                        round_body(trel, lead)
                    if trel > 0:
                        drip(trel - 1)
                GSTEP[0] = SCBASE[0] + TCsc
                if TAFAST:
                    for g in range(NCHAIN):
                        op_r1(g)
                for g in range(NCHAIN):
                    op_v(g, hch[g], TCsc - 1)
                while emitted < ncopies:
                    pending[emitted]()
                    emitted += 1
                return hch

            # main pipeline over chunks
            prev_v = None
            prev_tc = None
            prev_projs = None
            t0 = 0
            t0s = []
            for c, TCc in enumerate(chunks):
                if c == 0:
                    xa, x8 = first_x
                else:
                    xa = xpool.tile([128, DC, BL, TC], f16, tag="xa", name=f"xa_{c}")
                    x8 = xpool.tile(
                        [128, 2, DC, BL, TC], f8, tag="x8", name=f"x8_{c}"
                    )
                    for dc in range(DC):
                        nc.sync.dma_start(
                            xa[:, dc, :, :TCc], xA_r[dc, :, :, t0 : t0 + TCc]
                        )
                        for s in range(2):
                            nc.sync.dma_start(
                                x8[:, s, dc, :, :TCc],
                                x8_r[s, dc, :, :, t0 : t0 + TCc],
                            )
                projs, copies = emit_matmuls(c, TCc, xa, x8)
                if c == 0:
                    for do in copies:
                        do()
                else:
                    sc = c - 1
                    TCsc = chunks[sc]
                    hch = emit_scan(sc, TCsc, prev_projs, prev_v, prev_tc, copies)
                    for uh in range(UH):
                        for gi, (b0, b1) in enumerate(GROUPS):
                            nc.sync.dma_start(
                                outT_r[uh, :, b0:b1, t0s[sc] : t0s[sc] + TCsc],
                                hch[gi][:, uh, :, :TCsc],
                            )
                    prev_v = hch
                    prev_tc = TCsc
                prev_projs = projs
                t0s.append(t0)
                t0 += TCc
            sc = len(chunks) - 1
            TCsc = chunks[sc]
            hch = emit_scan(sc, TCsc, prev_projs, prev_v, prev_tc, [])
            # quarter the final chunk's writeback so it streams out behind
            # the scan instead of serializing after the last step
            QS = max(TCsc // 4, 1)
            for q0 in range(0, TCsc, QS):
                q1 = min(q0 + QS, TCsc)
                for uh in range(UH):
                    for gi, (b0, b1) in enumerate(GROUPS):
                        nc.sync.dma_start(
                            outT_r[uh, :, b0:b1, t0s[sc] + q0 : t0s[sc] + q1],
                            hch[gi][:, uh, :, q0:q1],
                        )

    nc.compile()
    return nc


def _get_nc(T_, TC, use_memory, use_bias):
    key = (T_, TC, use_memory, use_bias)
    if key not in _CACHE:
        _CACHE[key] = _build(T_, TC, use_memory, use_bias)
    return _CACHE[key]


def kernel(
    x,
    kernel_z,
    kernel_r,
    kernel_h,
    memory_z,
    memory_r,
    bias_z,
    bias_r,
    bias_h,
):
    from concourse import bass_utils

    x = np.asarray(x, dtype=np.float32)
    Ks = {
        "z": np.asarray(kernel_z, dtype=np.float32),
        "r": np.asarray(kernel_r, dtype=np.float32),
        "h": np.asarray(kernel_h, dtype=np.float32),
    }
    mem = {
        "z": np.asarray(memory_z, dtype=np.float32),
        "r": np.asarray(memory_r, dtype=np.float32),
    }
    bias = {
        "z": np.asarray(bias_z, dtype=np.float32),
        "r": np.asarray(bias_r, dtype=np.float32),
        "h": np.asarray(bias_h, dtype=np.float32),
    }

    B_, T_, D_ = x.shape
    assert (B_, D_) == (B, D), (x.shape,)
    TC = int(os.environ.get("BRU_TC", "32"))

    use_memory = not all(np.all(m == 1.0) for m in mem.values())
    use_bias = not all(np.all(b == 0.0) for b in bias.values())

    nc = _get_nc(T_, TC, use_memory, use_bias)

    import ml_dtypes

    f8e5 = ml_dtypes.float8_e5m2

    # Split weights once (shared across cores).  The z-gate weights/bias are
    # pre-halved: the kernel computes tau = tanh(0.5*zin) instead of
    # sigmoid(zin).  Each gate ships the fp16 main K1 plus a DoubleRow fp8
    # pair [K1*2^-4, K2*2^8] whose slot scalings cancel against the fp8
    # moving pair [e*2^4, A*2^-8].
    w1_full = {}
    w8_full = {}
    for g, K in Ks.items():
        if g == "z":
            K = K * np.float32(0.5)
        K1 = K.astype(np.float16)
        K2 = K - K1.astype(np.float32)
        k8 = np.empty((2, D, K.shape[1]), dtype=f8e5)
        k8[0] = (K1.astype(np.float32) * np.float32(2.0 ** -4)).astype(f8e5)
        k8[1] = (K2 * np.float32(2.0 ** 8)).astype(f8e5)
        w1_full[g] = K1
        w8_full[g] = k8

    in_maps = []
    for c in range(NCORES):
        bg, ug = divmod(c, NUG)
        xc = x[bg * BL : (bg + 1) * BL].reshape(BL * T_, D)
        xcT = np.ascontiguousarray(xc.T)  # [D, NTOK] fp32
        A = xcT.astype(np.float16)
        e = xcT - A.astype(np.float32)
        x8 = np.empty((2, D, xcT.shape[1]), dtype=f8e5)
        x8[0] = (e * np.float32(16.0)).astype(f8e5)
        x8[1] = (A.astype(np.float32) * np.float32(2.0 ** -8)).astype(f8e5)
        us = slice(ug * UHALF, (ug + 1) * UHALF)
        m = {"xA": A, "x8d": x8}
        for g in "zrh":
            m[f"k1{g}"] = np.ascontiguousarray(w1_full[g][:, us])
            m[f"k8{g}"] = np.ascontiguousarray(w8_full[g][:, :, us])
        if use_memory:
            # element (p, uh, b) = mem[ug*UHALF + uh*128 + p], pre-scaled
            for name, v, sc_ in (
                ("mzb", mem["z"], 0.25),
                ("mrb", mem["r"], 0.5),
            ):
                mv = (v[us] * np.float32(sc_)).reshape(UH, 128).T  # [128, UH]
                m[name] = np.ascontiguousarray(
                    np.broadcast_to(mv[:, :, None], (128, UH, BL))
                )
        if use_bias:
            for g in "zrh":
                bv = bias[g][us]
                if g == "z":
                    bv = bv * np.float32(0.5)
                m[f"bt{g}"] = np.ascontiguousarray(bv.reshape(UH, 128).T)
        in_maps.append(m)

    res = bass_utils.run_bass_kernel_spmd(nc, in_maps, core_ids=list(range(NCORES)))

    out = np.empty((B, T_, U), dtype=np.float32)
    for c in range(NCORES):
        bg, ug = divmod(c, NUG)
        oT = res.results[c]["outT"]  # [UHALF, BL*T_] holding v = 2h
        out[bg * BL : (bg + 1) * BL, :, ug * UHALF : (ug + 1) * UHALF] = (
            oT.reshape(UHALF, BL, T_).transpose(1, 2, 0)
        )
    out *= np.float32(0.5)
    return out



# revision 52
# speedup vs baseline: 1.2847x; 1.0011x over previous
"""BRU (bistable recurrent unit) cell kernel for 8 Trainium2 NeuronCores.

Hardcoded problem: B=64, T=512, D=1024, U=1024, fp32.

Sharding: 8 cores = 4 batch-groups (16 batches each) x 2 unit-groups
(512 units each).  Per core the three input projections
    projT[u, token] = K[d,u].T @ xT[d, token],   token = b*512 + t
run on the PE as a 1.5-pass split:
    x @ K  =  A@K1  +  (e*2^4)@(K1*2^-4) + (A*2^-8)@(K2*2^8)
with A = fp16(x) (exact residual e = x - A in fp32) and K1 = fp16(K),
K2 = K - K1.  The main term is one fp16 matmul (1 cycle/row); the two
correction products run as ONE fp8e5 DoubleRow matmul (0.5 cycles/row,
both slot products summed in-PE), accumulating into the same fp32 PSUM
group.  Power-of-two slot scalings cancel exactly, so each slot product
lands unscaled; fp8 rounding only perturbs the (already ~2^-11) residual
terms, leaving ~1e-4 projection error at 1.5x one pass's PE cost.

The 512-step recurrence is elementwise with u on partitions, split into
two batch-group chains.  The wall-clock is 512 x the per-chain serial
step latency, so the scan is latency-engineered end to end:

- Sigmoid is re-expressed via tau = tanh(0.5 zin); t1 and tau get
  SEPARATE Tanh instructions so only t1 sits on the serial path.
- The next step's tanh inputs skip v' ("TAFAST"): with
  v' = 0.5 w2 - r1, ta(t+1) = (0.25 w2 + xr) - 0.5 r1, where the w2
  half precomputes off-path a whole act-visit early, leaving only
  r1 -> ta on the tail; v' itself (the stored output) is written
  off-path.  tb(t+1) reads the freshly written v' directly ("TBOLD").
- The Tile scheduler's internal cost model has no semaphore/ack
  latencies (the Act engine's SBUF-ack alone is ~185ns charged to every
  cross-engine consumer), so its greedy order stalls under the real
  timeline.  tile_wait_until gates pin every scan op to a hand-packed
  per-round template (CYC ns per step, chain 1 offset by OFF) that
  keeps both DVE (16 ops/round) and Act (6 tanh/round) ~90% busy with
  the serial path threading through the gaps.  Gates only shape the
  compile-time schedule; the emitted program carries no extra waits.
- PSUM->SBUF projection copies are split in half so each piece fits the
  Act engine's per-round idle window, and the last chunk's writeback is
  quartered so it streams out behind the scan.

Steady-state round: ~1.76us for 2 chains x 1 step (vs ~2.4us for the
naive emission); total ~1.00ms vs the 1.21ms 3-pass baseline.
"""

import os

import numpy as np

B, T, D, U = 64, 512, 1024, 1024
NCORES = 8
NBG = 4  # batch groups
NUG = 2  # unit groups
BL = B // NBG  # 16 batches per core
UHALF = U // NUG  # 512 units per core
UH = UHALF // 128  # 4 u-chunks

_CACHE: dict = {}


def _build(T_, TC, use_memory, use_bias):
    """Build and compile the per-core Bass program."""
    import concourse.mybir as mybir
    from concourse import bacc
    from concourse.tile import TileContext

    f32 = mybir.dt.float32
    f16 = mybir.dt.float16
    f8 = mybir.dt.float8e5
    Alu = mybir.AluOpType
    Act = mybir.ActivationFunctionType
    DR = mybir.MatmulPerfMode.DoubleRow

    NTOK = BL * T_
    NCH = T_ // TC
    DC = D // 128  # 8 d-chunks

    nc = bacc.Bacc("TRN2", target_bir_lowering=False, debug=False)

    xA = nc.dram_tensor("xA", [D, NTOK], f16, kind="ExternalInput").ap()
    x8d = nc.dram_tensor("x8d", [2, D, NTOK], f8, kind="ExternalInput").ap()
    k1 = {}
    k8 = {}
    for g in "zrh":
        k1[g] = nc.dram_tensor(f"k1{g}", [D, UHALF], f16, kind="ExternalInput").ap()
        k8[g] = nc.dram_tensor(f"k8{g}", [2, D, UHALF], f8, kind="ExternalInput").ap()
    if use_memory:
        mzb = nc.dram_tensor("mzb", [128, UH, BL], f32, kind="ExternalInput").ap()
        mrb = nc.dram_tensor("mrb", [128, UH, BL], f32, kind="ExternalInput").ap()
    if use_bias:
        bts = {
            g: nc.dram_tensor(f"bt{g}", [128, UH], f32, kind="ExternalInput").ap()
            for g in "zrh"
        }
    outT = nc.dram_tensor("outT", [UHALF, NTOK], f32, kind="ExternalOutput").ap()

    xA_r = xA.rearrange("(dc p) (b t) -> dc p b t", dc=DC, b=BL)
    x8_r = x8d.rearrange("two (dc p) (b t) -> two dc p b t", dc=DC, b=BL)
    outT_r = outT.rearrange("(uh p) (b t) -> uh p b t", uh=UH, b=BL)

    # Chunk schedule: optional short prefix chunks let the scan's first
    # rounds start as soon as a few projected columns exist instead of
    # waiting out a full TC-column matmul block.
    prefix = [
        int(p) for p in os.environ.get("BRU_PREFIX", "").split("+") if p
    ]
    assert all(0 < p <= TC for p in prefix), prefix
    rest = T_ - sum(prefix)
    assert rest % TC == 0, (prefix, T_)
    chunks = prefix + [TC] * (rest // TC)
    assert sum(chunks) == T_, (chunks, T_)

    with TileContext(nc) as tc:
        with (
            tc.tile_pool(name="weights", bufs=1) as wpool,
            tc.tile_pool(name="xin", bufs=2) as xpool,
            tc.tile_pool(name="proj", bufs=2) as ppool,
            tc.tile_pool(name="hout", bufs=3) as hpool,
            tc.tile_pool(name="tmp", bufs=int(os.environ.get("BRU_SBUFS", "20"))) as spool,
            tc.tile_pool(name="misc", bufs=1) as mpool,
            tc.tile_pool(name="psum", bufs=8, space="PSUM") as qpool,
        ):
            # Startup order: z-gate weights, then the first x chunk, then
            # the remaining weights, so the PE's first PSUM group can start
            # as early as possible.
            TC0 = chunks[0]
            w1 = {}
            w8 = {}
            for g in "zrh":
                w1[g] = wpool.tile([128, DC, UHALF], f16, tag=f"w1{g}", name=f"w1{g}")
                w8[g] = wpool.tile(
                    [128, 2, DC, UHALF], f8, tag=f"w8{g}", name=f"w8{g}"
                )
            nc.sync.dma_start(
                w1["z"][:, :, :], k1["z"].rearrange("(dc p) u -> p dc u", p=128)
            )
            xa = xpool.tile([128, DC, BL, TC], f16, tag="xa", name="xa_0")
            x8 = xpool.tile([128, 2, DC, BL, TC], f8, tag="x8", name="x8_0")
            for dc in range(DC):
                nc.sync.dma_start(xa[:, dc, :, :TC0], xA_r[dc, :, :, 0:TC0])
            for s in range(2):
                nc.sync.dma_start(
                    w8["z"][:, s, :, :],
                    k8["z"][s].rearrange("(dc p) u -> p dc u", p=128),
                )
            for dc in range(DC):
                for s in range(2):
                    nc.sync.dma_start(
                        x8[:, s, dc, :, :TC0], x8_r[s, dc, :, :, 0:TC0]
                    )
            first_x = (xa, x8)
            for g in "rh":
                nc.sync.dma_start(
                    w1[g][:, :, :], k1[g].rearrange("(dc p) u -> p dc u", p=128)
                )
                for s in range(2):
                    nc.sync.dma_start(
                        w8[g][:, s, :, :],
                        k8[g][s].rearrange("(dc p) u -> p dc u", p=128),
                    )
            if use_memory:
                # host passes mzb = 0.25*m_z, mrb = 0.5*m_r broadcasts
                mz4_t = mpool.tile([128, UH, BL], f32, tag="mz4", name="mz4")
                mr2_t = mpool.tile([128, UH, BL], f32, tag="mr2", name="mr2")
                nc.sync.dma_start(mz4_t[:, :, :], mzb[:, :, :])
                nc.sync.dma_start(mr2_t[:, :, :], mrb[:, :, :])
            if use_bias:
                b_t = {}
                for g in "zrh":
                    b_t[g] = mpool.tile([128, UH], f32, tag=f"b{g}", name=f"b{g}")
                    nc.sync.dma_start(b_t[g][:, :], bts[g][:, :])

            # ---------------------------------------------------------
            # Software-pipelined scan over two batch-group chains.
            #
            # Math (per step, with carried state v = 2h):
            #   t1  = tanh(h*m_r + xr)
            #   tau = tanh(0.5*(h*m_z + xz)) so  1-z = 0.5*(1-tau)
            #   hh  = tanh(xh + (t1+1)*h)
            #   v'  = 2h' = (v/2 + hh) + tau*(v/2 - hh)
            # The host folds 0.5 into the z-gate weights/bias and halves the
            # output, so the kernel stores v.
            #
            # The wall-clock of the scan is 512 x the per-chain serial step
            # latency (chains are batch splits; each runs all T steps), so
            # the emission is latency-driven: t1 and tau get SEPARATE Tanh
            # instructions so that only t1 sits on the serial path
            # v' -> t1in -> t1 -> w -> hin -> hh -> r1 -> v'; tau's Tanh and
            # the w2 blend ride in the act/DVE slack between path ops.  The
            # Activation engine's SBUF-ack (~185ns) is charged to every
            # cross-engine consumer, so each removed act visit saves ~450ns
            # of path.
            # ---------------------------------------------------------
            NCHAIN = int(os.environ.get("BRU_NCHAIN", "2"))
            if NCHAIN == 3:
                HBS = [6, 5, 5]
            else:
                HBS = [BL // NCHAIN] * NCHAIN
            bnds = [0]
            for hb in HBS:
                bnds.append(bnds[-1] + hb)
            GROUPS = tuple((bnds[i], bnds[i + 1]) for i in range(NCHAIN))

            v0t = []
            for gi in range(NCHAIN):
                vg = mpool.tile([128, UH, HBS[gi]], f32, tag=f"v0{gi}",
                                name=f"v0{gi}")
                nc.gpsimd.memset(vg[:, :, :], 0.0)
                v0t.append(vg)

            def tmp(tag, gi, shape=None):
                return spool.tile(shape or [128, UH, HBS[gi]], f32,
                                  tag=f"{tag}{gi}", name=f"{tag}{gi}")

            XMODE = os.environ.get("BRU_XMODE", "split")
            R1POS = os.environ.get("BRU_R1POS", "late")
            ALT_LEAD = os.environ.get("BRU_ALT_LEAD", "0") == "1"
            TAFAST = os.environ.get("BRU_TAFAST", "1") == "1" and not use_memory
            TBOLD = os.environ.get("BRU_TBOLD", "1") == "1"

            # Manual schedule gates (tile_wait_until): the Tile scheduler's
            # internal cost model has no sem/ack latencies, so its greedy
            # order stalls under the real timeline (e.g. it queues the other
            # chain's r1 ahead of a ready ta).  Gates pin each scan op to a
            # hand-packed per-round template instead; they only constrain the
            # compile-time schedule, not the hardware program.
            GATES = os.environ.get("BRU_GATES", "1") == "1"
            CYC = float(os.environ.get("BRU_CYC", "3500"))
            OFF = float(os.environ.get("BRU_OFF", "480"))
            GT0 = float(os.environ.get("BRU_T0", "40000"))
            G_OP = {
                "t1": 0.0, "tau": 212.0, "hh": 887.0,
                "w": 430.0, "hin": 619.0, "w2": 713.0,
                "taw": 910.0, "tbw": float(os.environ.get("BRU_G_TBW", "1004")), "r1": 1317.0,
                "ta": 1506.0,
                "tb": 1694.0 if os.environ.get("BRU_TBOLD", "1") == "1"
                else 1600.0,
                "v": 1600.0 if os.environ.get("BRU_TBOLD", "1") == "1"
                else 1694.0,
                "copy": 1523.0,
            }
            for kv in os.environ.get("BRU_GOP", "").split("+"):
                if kv:
                    k, _, v = kv.partition("=")
                    G_OP[k] = float(v)
            import contextlib

            def gat(op, s, gi=0):
                if not GATES:
                    return contextlib.nullcontext()
                t = GT0 + s * CYC + gi * OFF + G_OP[op]
                return tc.tile_wait_until(t / 1e6)

            SCBASE = [0]
            GSTEP = [0]

            state = [dict() for _ in range(NCHAIN)]
            fin = [None] * NCHAIN

            def op_v(gi, hch_g, trel):
                # v' = 0.5*w2 - (tau-1)*hh closes step trel of chain gi
                s = fin[gi]
                with gat("v", SCBASE[0] + trel, gi):
                    nc.vector.scalar_tensor_tensor(
                        hch_g[:, :, :, trel], s["w2"][:, :, :], 0.5,
                        s["r1"][:, :, :], Alu.mult, Alu.subtract,
                    )

            def stage_F(gi, v, pz, pr, trel):
                """a = t1in = h*m_r + xr;  b = 0.5*zin = h*mz/2 + xz/2.
                (xz/2 is pre-folded into the z projection host-side.)"""
                s = state[gi] = {}
                b0, b1 = GROUPS[gi]
                xr_t = pr[:, :, b0:b1, trel]
                xzh_t = pz[:, :, b0:b1, trel]
                if XMODE == "merged":
                    stg = tmp("stg", gi, [128, 2, UH, HBS[gi]])
                    s["ta"] = stg[:, 0]
                    s["tb"] = stg[:, 1]
                    s["stg"] = stg
                elif XMODE == "htau":
                    # tb shares a tile with hin: ONE Tanh at H-time yields
                    # both hh and tau, cutting the act count to 4/round while
                    # tau (only needed by the post-H blend) stays off the
                    # t1 serial path.
                    hb = tmp("hb", gi, [128, 2, UH, HBS[gi]])
                    s["ta"] = tmp("ta", gi)[:, :, :]
                    s["tb"] = hb[:, 1]
                    s["hb"] = hb
                else:
                    s["ta"] = tmp("ta", gi)[:, :, :]
                    s["tb"] = tmp("tb", gi)[:, :, :]
                if use_memory:
                    hm_r = tmp("hmr", gi)
                    hm_z = tmp("hmz", gi)
                    nc.vector.tensor_mul(hm_r[:, :, :], v, mr2_t[:, :, b0:b1])
                    nc.vector.tensor_add(s["ta"], hm_r[:, :, :], xr_t)
                    nc.vector.tensor_mul(hm_z[:, :, :], v, mz4_t[:, :, b0:b1])
                    nc.vector.tensor_add(s["tb"], hm_z[:, :, :], xzh_t)
                else:
                    with gat("ta", GSTEP[0] - 1, gi):
                        nc.vector.scalar_tensor_tensor(
                            s["ta"], v, 0.5, xr_t, Alu.mult, Alu.add
                        )
                    with gat("tb", GSTEP[0] - 1, gi):
                        nc.vector.scalar_tensor_tensor(
                            s["tb"], v, 0.25, xzh_t, Alu.mult, Alu.add
                        )
                s["v"] = v

            def stage_X(gi):
                s = state[gi]
                if XMODE == "merged":
                    sto = tmp("sto", gi, [128, 2, UH, HBS[gi]])
                    nc.scalar.activation(sto[:, :, :, :], s["stg"][:, :, :, :],
                                         Act.Tanh)
                    s["t1"] = sto[:, 0]
                    s["tau"] = sto[:, 1]
                elif XMODE == "htau":
                    s["t1"] = tmp("t1", gi)[:, :, :]
                    nc.scalar.activation(s["t1"], s["ta"], Act.Tanh)
                else:
                    s["t1"] = tmp("t1", gi)[:, :, :]
                    s["tau"] = tmp("tau", gi)[:, :, :]
                    with gat("t1", GSTEP[0], gi):
                        nc.scalar.activation(s["t1"], s["ta"], Act.Tanh)
                    with gat("tau", GSTEP[0], gi):
                        nc.scalar.activation(s["tau"], s["tb"], Act.Tanh)

            def stage_M(gi, ph, trel):
                # w2 sits between w -> hin so the same-engine RAW sem lag of
                # w is hidden behind w2's execution
                s = state[gi]
                b0, b1 = GROUPS[gi]
                xh_t = ph[:, :, b0:b1, trel]
                w = tmp("w", gi)
                with gat("w", GSTEP[0], gi):
                    nc.vector.scalar_tensor_tensor(
                        w[:, :, :], s["t1"], 1.0, s["v"], Alu.add, Alu.mult
                    )
                if XMODE == "htau":
                    s["hin"] = s["hb"][:, 0]
                else:
                    s["w2"] = tmp("w2", gi)
                    with gat("w2", GSTEP[0], gi):
                        nc.vector.scalar_tensor_tensor(
                            s["w2"][:, :, :], s["tau"], 1.0, s["v"],
                            Alu.add, Alu.mult,
                        )
                    s["hin"] = tmp("hin", gi)[:, :, :]
                with gat("hin", GSTEP[0], gi):
                    nc.vector.scalar_tensor_tensor(
                        s["hin"], w[:, :, :], 0.5, xh_t, Alu.mult, Alu.add
                    )

            def stage_H(gi):
                s = state[gi]
                if XMODE == "htau":
                    so = tmp("so", gi, [128, 2, UH, HBS[gi]])
                    nc.scalar.activation(so[:, :, :, :], s["hb"][:, :, :, :],
                                         Act.Tanh)
                    s["hh"] = so[:, 0]
                    s["tau"] = so[:, 1]
                else:
                    s["hh"] = tmp("hh", gi)[:, :, :]
                    with gat("hh", GSTEP[0], gi):
                        nc.scalar.activation(s["hh"], s["hin"], Act.Tanh)

            def op_w2(gi):
                s = state[gi]
                s["w2"] = tmp("w2", gi)
                nc.vector.scalar_tensor_tensor(
                    s["w2"][:, :, :], s["tau"], 1.0, s["v"], Alu.add, Alu.mult
                )

            def op_r1(gi):
                s = state[gi]
                s["r1"] = tmp("r1", gi)
                with gat("r1", GSTEP[0] - 1, gi):
                    nc.vector.scalar_tensor_tensor(
                        s["r1"][:, :, :], s["tau"], 1.0, s["hh"],
                        Alu.subtract, Alu.mult,
                    )

            # --- TAFAST path: the next step's tanh inputs skip v'. ---
            # ta(t+1) = 0.5*v'(t) + xr = (0.25*w2 + xr) - 0.5*r1, and the
            # (0.25*w2 + xr) half ("taw") only needs w2 — available a whole
            # act-visit earlier than v' — so the serial tail after hh is just
            # r1 -> ta instead of r1 -> v' -> ta.  v' itself (the stored
            # output) drops off the serial path entirely.
            def op_taw(gi, pz, pr, trel):
                s = state[gi]
                b0, b1 = GROUPS[gi]
                s["taw"] = tmp("taw", gi)
                with gat("taw", GSTEP[0] - 1, gi):
                    nc.vector.scalar_tensor_tensor(
                        s["taw"][:, :, :], s["w2"][:, :, :], 0.25,
                        pr[:, :, b0:b1, trel], Alu.mult, Alu.add,
                    )
                if not TBOLD:
                    s["tbw"] = tmp("tbw", gi)
                    with gat("tbw", GSTEP[0] - 1, gi):
                        nc.vector.scalar_tensor_tensor(
                            s["tbw"][:, :, :], s["w2"][:, :, :], 0.125,
                            pz[:, :, b0:b1, trel], Alu.mult, Alu.add,
                        )

            def stage_F_fast(gi, vnext, pz=None, trel=None):
                # consumes prev step's r1 + taw/tbw; v for the w/w2 ops is
                # the hch slice written by the (off-path) op_v
                pv = fin[gi]
                s = state[gi] = {}
                s["ta"] = tmp("ta", gi)[:, :, :]
                s["tb"] = tmp("tb", gi)[:, :, :]
                with gat("ta", GSTEP[0] - 1, gi):
                    nc.vector.scalar_tensor_tensor(
                        s["ta"], pv["r1"][:, :, :], -0.5, pv["taw"][:, :, :],
                        Alu.mult, Alu.add,
                    )
                if TBOLD:
                    b0, b1 = GROUPS[gi]
                    with gat("tb", GSTEP[0] - 1, gi):
                        nc.vector.scalar_tensor_tensor(
                            s["tb"], vnext, 0.25, pz[:, :, b0:b1, trel],
                            Alu.mult, Alu.add,
                        )
                else:
                    with gat("tb", GSTEP[0] - 1, gi):
                        nc.vector.scalar_tensor_tensor(
                            s["tb"], pv["r1"][:, :, :], -0.25,
                            pv["tbw"][:, :, :], Alu.mult, Alu.add,
                        )
                s["v"] = vnext

            def emit_matmuls(c, TCc, xa, x8):
                projs = {}
                copies = []
                for g in "zrh":
                    pg = ppool.tile(
                        [128, UH, BL, TC], f32, tag=f"p{g}", name=f"p{g}_{c}"
                    )
                    projs[g] = pg
                    for uh in range(UH):
                        us = slice(uh * 128, (uh + 1) * 128)
                        ps = qpool.tile([128, BL, TC], f32, tag="ps")
                        for dc in range(DC):
                            nc.tensor.matmul(
                                ps[:, :, :TCc], w1[g][:, dc, us], xa[:, dc, :, :TCc],
                                start=(dc == 0), stop=False,
                            )
                        for dc in range(DC):
                            nc.tensor.matmul(
                                ps[:, :, :TCc], w8[g][:, :, dc, us],
                                x8[:, :, dc, :, :TCc],
                                start=False, stop=(dc == DC - 1), perf_mode=DR,
                            )

                        copy_eng = os.environ.get("BRU_COPY_ENG", "act")
                        csplit = int(os.environ.get("BRU_COPY_SPLIT", "2"))
                        if csplit == 3:
                            bcuts = [0, 6, 11, 16]
                        else:
                            bcuts = [BL // csplit * j for j in range(csplit + 1)]

                        def mkcopy(pg=pg, uh=uh, ps=ps, g=g, ci=len(copies),
                                   copy_eng=copy_eng, b0=0, b1=BL):
                            def do():
                                if use_bias:
                                    # bias fold needs the act engine
                                    nc.scalar.activation(
                                        pg[:, uh, b0:b1, :TCc],
                                        ps[:, b0:b1, :TCc],
                                        Act.Identity, bias=b_t[g][:, uh : uh + 1],
                                    )
                                elif copy_eng == "act" or (
                                    copy_eng == "alt" and ci % 2 == 0
                                ):
                                    # Pool can't read PSUM and DMA can't source
                                    # it, so the drains go on the two queues
                                    # that can reach PSUM.  Split into pieces
                                    # small enough to fit the act engine's
                                    # per-round idle window.
                                    nc.scalar.activation(
                                        pg[:, uh, b0:b1, :TCc],
                                        ps[:, b0:b1, :TCc],
                                        Act.Identity,
                                    )
                                else:
                                    nc.vector.tensor_copy(
                                        pg[:, uh, b0:b1, :TCc],
                                        ps[:, b0:b1, :TCc],
                                    )
                            return do

                        for j in range(csplit):
                            copies.append(
                                mkcopy(b0=bcuts[j], b1=bcuts[j + 1])
                            )
                return projs, copies

            def emit_scan(sc, TCsc, projs, prev_v, prev_tc, pending):
                pz, pr, ph = projs["z"], projs["r"], projs["h"]
                hch = [
                    hpool.tile([128, UH, HBS[gi], TC], f32, tag=f"hch{gi}",
                               name=f"hch{gi}_{sc}")
                    for gi in range(NCHAIN)
                ]

                def v_of(gi, trel):
                    if trel == 0:
                        if sc == 0:
                            return v0t[gi][:, :, :]
                        return prev_v[gi][:, :, :, prev_tc - 1]
                    return hch[gi][:, :, :, trel - 1]

                ncopies = len(pending)
                emitted = 0

                def drip(trel):
                    nonlocal emitted
                    want = ((trel + 1) * ncopies) // max(TCsc - 1, 1)
                    while emitted < min(want, ncopies):
                        with gat("copy", SCBASE[0] + trel, 0):
                            pending[emitted]()
                        emitted += 1

                def round_body(trel, lead, emit_r1=True):
                    order = [(lead + i) % NCHAIN for i in range(NCHAIN)]
                    if trel > 0:
                        for g in order:
                            op_v(g, hch[g], trel - 1)
                    for g in order:
                        stage_F(g, v_of(g, trel), pz, pr, trel)
                        stage_X(g)
                    if R1POS == "inline" and emit_r1:
                        for g in order:
                            stage_M(g, ph, trel)
                            stage_H(g)
                            if XMODE == "htau":
                                op_w2(g)
                            op_r1(g)
                    else:
                        for g in order:
                            stage_M(g, ph, trel)
                            stage_H(g)
                        for g in order:
                            if XMODE == "htau":
                                op_w2(g)
                            if emit_r1:
                                op_r1(g)
                    for g in order:
                        fin[g] = state[g]

                def round_body_fast(trel, lead):
                    # steady-state round: r1/ta/tb of this step chain off the
                    # previous H directly; v' is written off-path
                    order = [(lead + i) % NCHAIN for i in range(NCHAIN)]
                    if trel == 0:
                        round_body(0, lead, emit_r1=False)
                        return
                    for g in order:
                        op_taw(g, pz, pr, trel)
                    for g in order:
                        op_r1(g)
                        if TBOLD:
                            op_v(g, hch[g], trel - 1)
                            stage_F_fast(g, v_of(g, trel), pz, trel)
                        else:
                            stage_F_fast(g, v_of(g, trel), pz, trel)
                            op_v(g, hch[g], trel - 1)
                        stage_X(g)
                    for g in order:
                        stage_M(g, ph, trel)
                        stage_H(g)
                    for g in order:
                        fin[g] = state[g]

                SCBASE[0] = t0s[sc]
                for trel in range(TCsc):
                    GSTEP[0] = SCBASE[0] + trel
                    lead = (trel % NCHAIN) if ALT_LEAD else 0
                    if TAFAST:
                        round_body_fast(trel, lead)
                    else:
                        round_body(trel, lead)
                    if trel > 0:
                        drip(trel - 1)
                GSTEP[0] = SCBASE[0] + TCsc
                if TAFAST:
                    for g in range(NCHAIN):
                        op_r1(g)
                for g in range(NCHAIN):
                    op_v(g, hch[g], TCsc - 1)
                while emitted < ncopies:
                    pending[emitted]()
                    emitted += 1
                return hch

            # main pipeline over chunks
            prev_v = None
            prev_tc = None
            prev_projs = None
            t0 = 0
            t0s = []
            for c, TCc in enumerate(chunks):
                if c == 0:
                    xa, x8 = first_x
                else:
                    xa = xpool.tile([128, DC, BL, TC], f16, tag="xa", name=f"xa_{c}")
                    x8 = xpool.tile(
                        [128, 2, DC, BL, TC], f8, tag="x8", name=f"x8_{c}"
                    )
                    for dc in range(DC):
                        nc.sync.dma_start(
                            xa[:, dc, :, :TCc], xA_r[dc, :, :, t0 : t0 + TCc]
                        )
                        for s in range(2):
                            nc.sync.dma_start(
                                x8[:, s, dc, :, :TCc],
                                x8_r[s, dc, :, :, t0 : t0 + TCc],
                            )
                projs, copies = emit_matmuls(c, TCc, xa, x8)
                if c == 0:
                    for do in copies:
                        do()
                else:
                    sc = c - 1
                    TCsc = chunks[sc]
                    hch = emit_scan(sc, TCsc, prev_projs, prev_v, prev_tc, copies)
                    for uh in range(UH):
                        for gi, (b0, b1) in enumerate(GROUPS):
                            nc.sync.dma_start(
                                outT_r[uh, :, b0:b1, t0s[sc] : t0s[sc] + TCsc],
                                hch[gi][:, uh, :, :TCsc],
                            )
                    prev_v = hch
                    prev_tc = TCsc
                prev_projs = projs
                t0s.append(t0)
                t0 += TCc
            sc = len(chunks) - 1
            TCsc = chunks[sc]
            hch = emit_scan(sc, TCsc, prev_projs, prev_v, prev_tc, [])
            # quarter the final chunk's writeback so it streams out behind
            # the scan instead of serializing after the last step
            QS = max(TCsc // 4, 1)
            for q0 in range(0, TCsc, QS):
                q1 = min(q0 + QS, TCsc)
                for uh in range(UH):
                    for gi, (b0, b1) in enumerate(GROUPS):
                        nc.sync.dma_start(
                            outT_r[uh, :, b0:b1, t0s[sc] + q0 : t0s[sc] + q1],
                            hch[gi][:, uh, :, q0:q1],
                        )

    nc.compile()
    return nc


def _get_nc(T_, TC, use_memory, use_bias):
    key = (T_, TC, use_memory, use_bias)
    if key not in _CACHE:
        _CACHE[key] = _build(T_, TC, use_memory, use_bias)
    return _CACHE[key]


def kernel(
    x,
    kernel_z,
    kernel_r,
    kernel_h,
    memory_z,
    memory_r,
    bias_z,
    bias_r,
    bias_h,
):
    from concourse import bass_utils

    x = np.asarray(x, dtype=np.float32)
    Ks = {
        "z": np.asarray(kernel_z, dtype=np.float32),
        "r": np.asarray(kernel_r, dtype=np.float32),
        "h": np.asarray(kernel_h, dtype=np.float32),
    }
    mem = {
        "z": np.asarray(memory_z, dtype=np.float32),
        "r": np.asarray(memory_r, dtype=np.float32),
    }
    bias = {
        "z": np.asarray(bias_z, dtype=np.float32),
        "r": np.asarray(bias_r, dtype=np.float32),
        "h": np.asarray(bias_h, dtype=np.float32),
    }

    B_, T_, D_ = x.shape
    assert (B_, D_) == (B, D), (x.shape,)
    TC = int(os.environ.get("BRU_TC", "32"))

    use_memory = not all(np.all(m == 1.0) for m in mem.values())
    use_bias = not all(np.all(b == 0.0) for b in bias.values())

    nc = _get_nc(T_, TC, use_memory, use_bias)

    import ml_dtypes

    f8e5 = ml_dtypes.float8_e5m2

    # Split weights once (shared across cores).  The z-gate weights/bias are
    # pre-halved: the kernel computes tau = tanh(0.5*zin) instead of
    # sigmoid(zin).  Each gate ships the fp16 main K1 plus a DoubleRow fp8
    # pair [K1*2^-4, K2*2^8] whose slot scalings cancel against the fp8
    # moving pair [e*2^4, A*2^-8].
    w1_full = {}
    w8_full = {}
    for g, K in Ks.items():
        if g == "z":
            K = K * np.float32(0.5)
        K1 = K.astype(np.float16)
        K2 = K - K1.astype(np.float32)
        k8 = np.empty((2, D, K.shape[1]), dtype=f8e5)
        k8[0] = (K1.astype(np.float32) * np.float32(2.0 ** -4)).astype(f8e5)
        k8[1] = (K2 * np.float32(2.0 ** 8)).astype(f8e5)
        w1_full[g] = K1
        w8_full[g] = k8

    in_maps = []
    for c in range(NCORES):
        bg, ug = divmod(c, NUG)
        xc = x[bg * BL : (bg + 1) * BL].reshape(BL * T_, D)
        xcT = np.ascontiguousarray(xc.T)  # [D, NTOK] fp32
        A = xcT.astype(np.float16)
        e = xcT - A.astype(np.float32)
        x8 = np.empty((2, D, xcT.shape[1]), dtype=f8e5)
        x8[0] = (e * np.float32(16.0)).astype(f8e5)
        x8[1] = (A.astype(np.float32) * np.float32(2.0 ** -8)).astype(f8e5)
        us = slice(ug * UHALF, (ug + 1) * UHALF)
        m = {"xA": A, "x8d": x8}
        for g in "zrh":
            m[f"k1{g}"] = np.ascontiguousarray(w1_full[g][:, us])
            m[f"k8{g}"] = np.ascontiguousarray(w8_full[g][:, :, us])
        if use_memory:
            # element (p, uh, b) = mem[ug*UHALF + uh*128 + p], pre-scaled
            for name, v, sc_ in (
                ("mzb", mem["z"], 0.25),
                ("mrb", mem["r"], 0.5),
            ):
                mv = (v[us] * np.float32(sc_)).reshape(UH, 128).T  # [128, UH]
                m[name] = np.ascontiguousarray(
                    np.broadcast_to(mv[:, :, None], (128, UH, BL))
                )
        if use_bias:
            for g in "zrh":
                bv = bias[g][us]
                if g == "z":
                    bv = bv * np.float32(0.5)
                m[f"bt{g}"] = np.ascontiguousarray(bv.reshape(UH, 128).T)
        in_maps.append(m)

    res = bass_utils.run_bass_kernel_spmd(nc, in_maps, core_ids=list(range(NCORES)))

    out = np.empty((B, T_, U), dtype=np.float32)
    for c in range(NCORES):
        bg, ug = divmod(c, NUG)
        oT = res.results[c]["outT"]  # [UHALF, BL*T_] holding v = 2h
        out[bg * BL : (bg + 1) * BL, :, ug * UHALF : (ug + 1) * UHALF] = (
            oT.reshape(UHALF, BL, T_).transpose(1, 2, 0)
        )
    out *= np.float32(0.5)
    return out

